# revision 2
# baseline (speedup 1.0000x reference)
"""Trainium2 Bass kernel for ContinuousAttentiveStatisticsPooling.

Shape config (hardcoded): B=8, C=256, L=8192, A=128, 8 NeuronCores,
pure data parallel over B (one example per core).

Math restructure (per example, x is [C, L]):
  - Host zeroes x beyond the valid length -> all L-reductions over full L
    equal masked reductions (gmean/gstd; and W @ x has exact-zero tails).
  - gmean = sum(x)/total ; gstd = sqrt(clip(sum(x^2)/total - gmean^2))
  - values   = W1 @ x + cv,   cv      = W2 @ gmean + W3 @ gstd + b_val
  - pre_h    = Wt1 @ x + ch,  ch      = Wt2 @ gmean + Wt3 @ gstd + b_tdnn
  - h        = relu(pre_h)         (gamma folded into Wc, beta into b')
  - scores   = Wc' @ h + b',  Wc'     = (w_conv * gamma).T stuff, b' = w_conv@beta + b_conv
  - p        = exp(scores)  (no max subtraction needed; scores are O(1))
  - The invalid tail of scores is an exactly-computable constant s_inv, so
    Z_valid = sum_L p - n_invalid * exp(s_inv + b') ; p*vraw has zero tail.
  - amean = S1/Z + cv ; avar = S2/Z - (S1/Z)^2 with
    S1 = sum p*vraw, S2 = sum p*vraw^2 (vraw = W1 @ x, no bias).
"""

import sys

if "/opt/trn_rl_repo" not in sys.path:
    sys.path.insert(0, "/opt/trn_rl_repo")

import numpy as np

import concourse.bass as bass
import concourse.mybir as mybir
import concourse.tile as tile
from concourse.bass_utils import run_bass_kernel_spmd

B, C, L, A = 8, 256, 8192, 128
CB = C // 128          # 2 c-blocks
NCHUNK = 16            # streaming chunks over L
LC = L // NCHUNK       # 512
NDMA = 4               # x DMA chunks per c-block
LD = L // NDMA         # 2048
EPS = 1e-12
F32 = mybir.dt.float32
ALU = mybir.AluOpType
ACT = mybir.ActivationFunctionType

_mw_ctr = [0]


def _split_multiwaits(nc):
    """This walrus build supports only ONE sync-wait per instruction.
    Split multi-wait instructions into single-wait NoOps on the same engine
    (same-engine program order preserves semantics exactly)."""
    for f in nc.m.functions:
        for blk in f.blocks:
            insts = blk.instructions
            out = []
            changed = False
            for inst in insts:
                si = inst.sync_info
                if si is not None and len(si.on_wait) > 1:
                    changed = True
                    waits = list(si.on_wait)
                    for w in waits[:-1]:
                        _mw_ctr[0] += 1
                        nop = mybir.InstNoOp(
                            name=f"mwsplit-{_mw_ctr[0]}", ins=[], outs=[]
                        )
                        nop.engine = inst.engine
                        nop.sync_info = mybir.SyncInfo(on_wait=[w], on_update=[])
                        out.append(nop)
                    inst.sync_info = mybir.SyncInfo(
                        on_wait=[waits[-1]], on_update=list(si.on_update)
                    )
                out.append(inst)
            if changed:
                insts[:] = out


def _build_nc():
    nc = bass.Bass()
    x_d = nc.dram_tensor("x", [C, L], F32, kind="ExternalInput")
    wv1t_d = nc.dram_tensor("wv1t", [128, 2, CB, 128], F32, kind="ExternalInput")
    wcv_d = nc.dram_tensor("wcv", [128, 4, CB, 128], F32, kind="ExternalInput")
    wtt_d = nc.dram_tensor("wtt", [128, 2, 128], F32, kind="ExternalInput")
    wch_d = nc.dram_tensor("wch", [128, 4, 128], F32, kind="ExternalInput")
    wct_d = nc.dram_tensor("wct", [128, CB, 128], F32, kind="ExternalInput")
    bval_d = nc.dram_tensor("bval", [128, CB], F32, kind="ExternalInput")
    btdnn_d = nc.dram_tensor("btdnn", [128, 1], F32, kind="ExternalInput")
    bp_d = nc.dram_tensor("bp", [128, CB], F32, kind="ExternalInput")
    scal_d = nc.dram_tensor("scal", [128, 2], F32, kind="ExternalInput")
    out_d = nc.dram_tensor("out", [2 * C, 1], F32, kind="ExternalOutput")

    with tile.TileContext(nc) as tc:
        with (
            tc.tile_pool(name="consts", bufs=1) as cp,
            tc.tile_pool(name="xs", bufs=1) as xp,
            tc.tile_pool(name="hw", bufs=3) as hp,
            tc.tile_pool(name="pw", bufs=3) as pp,
            tc.tile_pool(name="pvw", bufs=3) as pvp,
            tc.tile_pool(name="pv2w", bufs=1) as pv2p,
        ):
            # ---- load weights / consts ----
            wv1t = cp.tile([128, 2, CB, 128], F32, tag="wv1t", name="wv1t")
            nc.sync.dma_start(out=wv1t, in_=wv1t_d[:, :, :, :])
            wcv = cp.tile([128, 4, CB, 128], F32, tag="wcv", name="wcv")
            nc.sync.dma_start(out=wcv, in_=wcv_d[:, :, :, :])
            wtt = cp.tile([128, 2, 128], F32, tag="wtt", name="wtt")
            nc.sync.dma_start(out=wtt, in_=wtt_d[:, :, :])
            wch = cp.tile([128, 4, 128], F32, tag="wch", name="wch")
            nc.sync.dma_start(out=wch, in_=wch_d[:, :, :])
            wct = cp.tile([128, CB, 128], F32, tag="wct", name="wct")
            nc.sync.dma_start(out=wct, in_=wct_d[:, :, :])
            bval = cp.tile([128, CB], F32, tag="bval", name="bval")
            nc.sync.dma_start(out=bval, in_=bval_d[:, :])
            btdnn = cp.tile([128, 1], F32, tag="btdnn", name="btdnn")
            nc.sync.dma_start(out=btdnn, in_=btdnn_d[:, :])
            bp = cp.tile([128, CB], F32, tag="bp", name="bp")
            nc.sync.dma_start(out=bp, in_=bp_d[:, :])
            scal = cp.tile([128, 2], F32, tag="scal", name="scal")
            nc.sync.dma_start(out=scal, in_=scal_d[:, :])

            # ---- load x (chunked) + stats accumulation ----
            xs = []
            sumxp = []
            sumsqp = []
            dummy1 = cp.tile([128, 1], F32, tag="dummy1", name="dummy1")
            for cb in range(CB):
                xs.append(xp.tile([128, L], F32, tag=f"x{cb}", name=f"x{cb}"))
                sumxp.append(cp.tile([128, NDMA], F32, tag=f"sumxp{cb}", name=f"sumxp{cb}"))
                sumsqp.append(cp.tile([128, NDMA], F32, tag=f"sumsqp{cb}", name=f"sumsqp{cb}"))
            for cb in range(CB):
                for j in range(NDMA):
                    sl = slice(j * LD, (j + 1) * LD)
                    nc.sync.dma_start(
                        out=xs[cb][:, sl], in_=x_d[cb * 128 : (cb + 1) * 128, sl]
                    )
                    nc.vector.tensor_scalar(
                        out=dummy1.broadcast_to((128, LD)),
                        in0=xs[cb][:, sl],
                        scalar1=1.0,
                        scalar2=0.0,
                        op0=ALU.mult,
                        op1=ALU.add,
                        accum_out=sumxp[cb][:, j : j + 1],
                    )
                    nc.scalar.activation(
                        out=dummy1.broadcast_to((128, LD)),
                        in_=xs[cb][:, sl],
                        func=ACT.Square,
                        accum_out=sumsqp[cb][:, j : j + 1],
                    )

            # ---- finalize stats: gmean / gstd per c-block ----
            gmean, gstd = [], []
            for cb in range(CB):
                sx = cp.tile([128, 1], F32, tag=f"sx{cb}", name=f"sx{cb}")
                nc.vector.tensor_reduce(
                    out=sx, in_=sumxp[cb], axis=mybir.AxisListType.X, op=ALU.add
                )
                sq = cp.tile([128, 1], F32, tag=f"sq{cb}", name=f"sq{cb}")
                nc.vector.tensor_reduce(
                    out=sq, in_=sumsqp[cb], axis=mybir.AxisListType.X, op=ALU.add
                )
                gm = cp.tile([128, 1], F32, tag=f"gm{cb}", name=f"gm{cb}")
                nc.vector.tensor_scalar_mul(out=gm, in0=sx, scalar1=scal[:, 0:1])
                msq = cp.tile([128, 1], F32, tag=f"msq{cb}", name=f"msq{cb}")
                nc.vector.tensor_scalar_mul(out=msq, in0=sq, scalar1=scal[:, 0:1])
                gm2 = cp.tile([128, 1], F32, tag=f"gm2{cb}", name=f"gm2{cb}")
                nc.vector.tensor_mul(out=gm2, in0=gm, in1=gm)
                gv = cp.tile([128, 1], F32, tag=f"gv{cb}", name=f"gv{cb}")
                nc.vector.tensor_sub(out=gv, in0=msq, in1=gm2)
                nc.vector.tensor_scalar_max(out=gv, in0=gv, scalar1=EPS)
                # gstd = exp(0.5*ln(gv)) -- keeps ACT on one table set (ln+exp)
                lng = cp.tile([128, 1], F32, tag=f"lng{cb}", name=f"lng{cb}")
                nc.scalar.activation(out=lng, in_=gv, func=ACT.Ln)
                gs = cp.tile([128, 1], F32, tag=f"gs{cb}", name=f"gs{cb}")
                nc.scalar.activation(out=gs, in_=lng, func=ACT.Exp, scale=0.5)
                gmean.append(gm)
                gstd.append(gs)

            gg = [gmean[0], gmean[1], gstd[0], gstd[1]]  # [f'] blocks of [gmean; gstd]

            # ---- derived consts via tiny PE matvecs ----
            with tc.tile_pool(name="pscst", bufs=2, space="PSUM") as pcst:
                ps_ch = pcst.tile([128, 1], F32, tag="pch", name="pch")
                for j in range(4):
                    nc.tensor.matmul(
                        ps_ch, lhsT=wch[:, j, :], rhs=gg[j], start=(j == 0), stop=(j == 3)
                    )
                const_h = cp.tile([128, 1], F32, tag="const_h", name="const_h")
                nc.scalar.activation(
                    out=const_h, in_=ps_ch, func=ACT.Identity, bias=btdnn[:, 0:1]
                )
                hinv = cp.tile([128, 1], F32, tag="hinv", name="hinv")
                nc.scalar.activation(out=hinv, in_=const_h, func=ACT.Relu)

                cv, pinv = [], []
                for cb in range(CB):
                    ps_cv = pcst.tile([128, 1], F32, tag="pcv", name="pcv")
                    for j in range(4):
                        nc.tensor.matmul(
                            ps_cv,
                            lhsT=wcv[:, j, cb, :],
                            rhs=gg[j],
                            start=(j == 0),
                            stop=(j == 3),
                        )
                    cvt = cp.tile([128, 1], F32, tag=f"cv{cb}", name=f"cv{cb}")
                    nc.scalar.activation(
                        out=cvt, in_=ps_cv, func=ACT.Identity, bias=bval[:, cb : cb + 1]
                    )
                    cv.append(cvt)
                    ps_si = pcst.tile([128, 1], F32, tag="psi", name="psi")
                    nc.tensor.matmul(ps_si, lhsT=wct[:, cb, :], rhs=hinv, start=True, stop=True)
                    pv_t = cp.tile([128, 1], F32, tag=f"pinv{cb}", name=f"pinv{cb}")
                    nc.scalar.activation(
                        out=pv_t, in_=ps_si, func=ACT.Exp, bias=bp[:, cb : cb + 1]
                    )
                    pinv.append(pv_t)

            # ---- streaming phase ----
            Zp = [cp.tile([128, NCHUNK], F32, tag=f"Zp{cb}", name=f"Zp{cb}") for cb in range(CB)]
            S1p = [cp.tile([128, NCHUNK], F32, tag=f"S1p{cb}", name=f"S1p{cb}") for cb in range(CB)]
            S2p = [cp.tile([128, NCHUNK], F32, tag=f"S2p{cb}", name=f"S2p{cb}") for cb in range(CB)]

            with (
                tc.tile_pool(name="psh", bufs=2, space="PSUM") as ps_h,
                tc.tile_pool(name="psv", bufs=4, space="PSUM") as ps_v,
                tc.tile_pool(name="pss", bufs=2, space="PSUM") as ps_s,
            ):
                for i in range(NCHUNK):
                    sl = slice(i * LC, (i + 1) * LC)
                    ph = ps_h.tile([128, LC], F32, tag="ph", name="ph")
                    nc.tensor.matmul(ph, lhsT=wtt[:, 0, :], rhs=xs[0][:, sl], start=True, stop=False)
                    nc.tensor.matmul(ph, lhsT=wtt[:, 1, :], rhs=xs[1][:, sl], start=False, stop=True)
                    h = hp.tile([128, LC], F32, tag="h", name="h")
                    nc.scalar.activation(out=h, in_=ph, func=ACT.Relu, bias=const_h[:, 0:1])
                    for cb in range(CB):
                        vps = ps_v.tile([128, LC], F32, tag="vraw", name="vraw")
                        nc.tensor.matmul(vps, lhsT=wv1t[:, 0, cb, :], rhs=xs[0][:, sl], start=True, stop=False)
                        nc.tensor.matmul(vps, lhsT=wv1t[:, 1, cb, :], rhs=xs[1][:, sl], start=False, stop=True)
                        sps = ps_s.tile([128, LC], F32, tag="scr", name="scr")
                        nc.tensor.matmul(sps, lhsT=wct[:, cb, :], rhs=h, start=True, stop=True)
                        p = pp.tile([128, LC], F32, tag="p", name="p")
                        nc.scalar.activation(
                            out=p, in_=sps, func=ACT.Exp,
                            bias=bp[:, cb : cb + 1],
                            accum_out=Zp[cb][:, i : i + 1],
                        )
                        pv = pvp.tile([128, LC], F32, tag="pv", name="pv")
                        nc.vector.scalar_tensor_tensor(
                            out=pv, in0=p, scalar=0.0, in1=vps,
                            op0=ALU.bypass, op1=ALU.mult,
                            accum_out=S1p[cb][:, i : i + 1],
                        )
                        pv2 = pv2p.tile([128, LC], F32, tag="pv2", name="pv2")
                        nc.vector.scalar_tensor_tensor(
                            out=pv2, in0=pv, scalar=0.0, in1=vps,
                            op0=ALU.bypass, op1=ALU.mult,
                            accum_out=S2p[cb][:, i : i + 1],
                        )

            # ---- finalize ----
            for cb in range(CB):
                Z = cp.tile([128, 1], F32, tag=f"Z{cb}", name=f"Z{cb}")
                nc.vector.tensor_reduce(out=Z, in_=Zp[cb], axis=mybir.AxisListType.X, op=ALU.add)
                corr = cp.tile([128, 1], F32, tag=f"corr{cb}", name=f"corr{cb}")
                nc.vector.tensor_scalar_mul(out=corr, in0=pinv[cb], scalar1=scal[:, 1:2])
                Zv = cp.tile([128, 1], F32, tag=f"Zv{cb}", name=f"Zv{cb}")
                nc.vector.tensor_sub(out=Zv, in0=Z, in1=corr)
                rz = cp.tile([128, 1], F32, tag=f"rz{cb}", name=f"rz{cb}")
                nc.vector.reciprocal(out=rz, in_=Zv)
                S1 = cp.tile([128, 1], F32, tag=f"S1{cb}", name=f"S1{cb}")
                nc.vector.tensor_reduce(out=S1, in_=S1p[cb], axis=mybir.AxisListType.X, op=ALU.add)
                S2 = cp.tile([128, 1], F32, tag=f"S2{cb}", name=f"S2{cb}")
                nc.vector.tensor_reduce(out=S2, in_=S2p[cb], axis=mybir.AxisListType.X, op=ALU.add)
                m1 = cp.tile([128, 1], F32, tag=f"m1{cb}", name=f"m1{cb}")
                nc.vector.tensor_scalar_mul(out=m1, in0=S1, scalar1=rz)
                amean = cp.tile([128, 1], F32, tag=f"amean{cb}", name=f"amean{cb}")
                nc.vector.tensor_add(out=amean, in0=m1, in1=cv[cb])
                t1 = cp.tile([128, 1], F32, tag=f"t1{cb}", name=f"t1{cb}")
                nc.vector.tensor_scalar_mul(out=t1, in0=S2, scalar1=rz)
                m1sq = cp.tile([128, 1], F32, tag=f"m1sq{cb}", name=f"m1sq{cb}")
                nc.vector.tensor_mul(out=m1sq, in0=m1, in1=m1)
                avar = cp.tile([128, 1], F32, tag=f"avar{cb}", name=f"avar{cb}")
                nc.vector.tensor_sub(out=avar, in0=t1, in1=m1sq)
                nc.vector.tensor_scalar_max(out=avar, in0=avar, scalar1=EPS)
                lnv = cp.tile([128, 1], F32, tag=f"lnv{cb}", name=f"lnv{cb}")
                nc.scalar.activation(out=lnv, in_=avar, func=ACT.Ln)
                astd = cp.tile([128, 1], F32, tag=f"astd{cb}", name=f"astd{cb}")
                nc.scalar.activation(out=astd, in_=lnv, func=ACT.Exp, scale=0.5)
                nc.sync.dma_start(out=out_d[cb * 128 : (cb + 1) * 128, :], in_=amean)
                nc.sync.dma_start(out=out_d[C + cb * 128 : C + (cb + 1) * 128, :], in_=astd)

    _split_multiwaits(nc)
    return nc


_NC_CACHE = None


def _get_nc():
    global _NC_CACHE
    if _NC_CACHE is None:
        _NC_CACHE = _build_nc()
    return _NC_CACHE


def _prep_inputs(x, lengths, w_val, b_val, w_tdnn, b_tdnn, bn_gamma, bn_beta,
                 w_conv, b_conv):
    x = np.asarray(x, dtype=np.float32)
    lengths = np.asarray(lengths, dtype=np.float32)
    w_val = np.asarray(w_val, dtype=np.float32)
    b_val = np.asarray(b_val, dtype=np.float32)
    w_tdnn = np.asarray(w_tdnn, dtype=np.float32)
    b_tdnn = np.asarray(b_tdnn, dtype=np.float32)
    bn_gamma = np.asarray(bn_gamma, dtype=np.float32)
    bn_beta = np.asarray(bn_beta, dtype=np.float32)
    w_conv = np.asarray(w_conv, dtype=np.float32)
    b_conv = np.asarray(b_conv, dtype=np.float32)

    mask = (np.arange(L, dtype=np.float32)[None, :] < (lengths * L)[:, None])
    total = mask.sum(axis=1).astype(np.float32)            # [B]
    xm = x * mask[:, None, :].astype(np.float32)           # zero the invalid tail

    def pack_lhsT(w, kblocks, cblocks):
        # w: [K, M] (contraction-major) -> [128, kblocks, cblocks, 128]
        Ktot, Mtot = w.shape
        assert Ktot == kblocks * 128 and Mtot == cblocks * 128
        return np.ascontiguousarray(
            w.reshape(kblocks, 128, cblocks, 128).transpose(1, 0, 2, 3)
        )

    W1T = w_val[:, :C].T                                   # [f, c]
    wv1t = pack_lhsT(W1T, 2, CB)
    Wcv = np.concatenate([w_val[:, C:2 * C].T, w_val[:, 2 * C:].T], axis=0)  # [2C, C]
    wcv = pack_lhsT(Wcv, 4, CB)
    WtT = w_tdnn[:, :C].T                                  # [f, a]
    wtt = pack_lhsT(WtT, 2, 1).reshape(128, 2, 128)
    Wch = np.concatenate([w_tdnn[:, C:2 * C].T, w_tdnn[:, 2 * C:].T], axis=0)
    wch = pack_lhsT(Wch, 4, 1).reshape(128, 4, 128)
    WcT = (w_conv * bn_gamma[None, :]).T                   # [a, c]
    wct = pack_lhsT(WcT, 1, CB).reshape(128, CB, 128)
    bprime = b_conv + w_conv @ bn_beta                     # [C]

    shared = {
        "wv1t": wv1t, "wcv": wcv, "wtt": wtt, "wch": wch, "wct": wct,
        "bval": np.ascontiguousarray(b_val.reshape(CB, 128).T),
        "btdnn": np.ascontiguousarray(b_tdnn.reshape(128, 1)),
        "bp": np.ascontiguousarray(bprime.reshape(CB, 128).T),
    }
    in_maps = []
    for b in range(B):
        m = dict(shared)
        m["x"] = np.ascontiguousarray(xm[b])
        scal = np.empty((128, 2), dtype=np.float32)
        scal[:, 0] = 1.0 / total[b]
        scal[:, 1] = L - total[b]
        m["scal"] = scal
        in_maps.append(m)
    return in_maps


def kernel(**inputs) -> np.ndarray:
    in_maps = _prep_inputs(**inputs)
    nc = _get_nc()
    res = run_bass_kernel_spmd(nc, in_maps, core_ids=list(range(B)))
    out = np.stack([res.results[b]["out"] for b in range(B)], axis=0)  # [B, 2C, 1]
    return out.astype(np.float32)


# revision 3
# speedup vs baseline: 1.3714x; 1.3714x over previous
"""Trainium2 Bass kernel for ContinuousAttentiveStatisticsPooling.

Shape config (hardcoded): B=8, C=256, L=8192, A=128, 8 NeuronCores,
pure data parallel over B (one example per core).

Math restructure (per example, x is [C, L]):
  - Host zeroes x beyond the valid length -> all L-reductions over full L
    equal masked reductions (gmean/gstd; and W @ x has exact-zero tails).
  - gmean = sum(x)/total ; gstd = sqrt(clip(sum(x^2)/total - gmean^2))
  - vraw     = W1 @ x  (no bias; amean/avar reconstructed from raw moments)
  - pre_h    = Wt1 @ x + ch,  ch = Wt2 @ gmean + Wt3 @ gstd + b_tdnn
  - h        = relu(pre_h)       (gamma folded into Wc', beta into b')
  - scores   = Wc' @ h + b'
  - p        = exp(scores)   (no max subtraction; scores are O(1))
  - The invalid tail of scores is an exactly-computable constant s_inv, so
    Z_valid = sum_L p - n_invalid * exp(s_inv + b') ; p*vraw has zero tail.
  - amean = S1/Z + cv ; avar = S2/Z - (S1/Z)^2 with
    S1 = sum p*vraw, S2 = sum p*vraw^2, cv = W2@gmean + W3@gstd + b_val.

Precision: x / matmul weights / elementwise interiors in bf16; all
reductions, biases, and the derived consts in fp32.
"""

import sys

if "/opt/trn_rl_repo" not in sys.path:
    sys.path.insert(0, "/opt/trn_rl_repo")

import numpy as np
import ml_dtypes

import concourse.bass as bass
import concourse.mybir as mybir
import concourse.tile as tile
from concourse.bass_utils import run_bass_kernel_spmd

B, C, L, A = 8, 256, 8192, 128
CB = C // 128          # 2 c-blocks
NCHUNK = 16            # streaming chunks over L
LC = L // NCHUNK       # 512
NDMA = 4               # x DMA chunks per c-block
LD = L // NDMA         # 2048
EPS = 1e-12
F32 = mybir.dt.float32
BF16 = mybir.dt.bfloat16
ALU = mybir.AluOpType
ACT = mybir.ActivationFunctionType

_mw_ctr = [0]


def _split_multiwaits(nc):
    """This walrus build supports only ONE sync-wait per instruction.
    Split multi-wait instructions into single-wait NoOps on the same engine
    (same-engine program order preserves semantics exactly)."""
    for f in nc.m.functions:
        for blk in f.blocks:
            insts = blk.instructions
            out = []
            changed = False
            for inst in insts:
                si = inst.sync_info
                if si is not None and len(si.on_wait) > 1:
                    changed = True
                    waits = list(si.on_wait)
                    for w in waits[:-1]:
                        _mw_ctr[0] += 1
                        nop = mybir.InstNoOp(
                            name=f"mwsplit-{_mw_ctr[0]}", ins=[], outs=[]
                        )
                        nop.engine = inst.engine
                        nop.sync_info = mybir.SyncInfo(on_wait=[w], on_update=[])
                        out.append(nop)
                    inst.sync_info = mybir.SyncInfo(
                        on_wait=[waits[-1]], on_update=list(si.on_update)
                    )
                out.append(inst)
            if changed:
                insts[:] = out


def _build_nc():
    nc = bass.Bass()
    x_d = nc.dram_tensor("x", [C, L], BF16, kind="ExternalInput")
    wv1t_d = nc.dram_tensor("wv1t", [128, 2, CB, 128], BF16, kind="ExternalInput")
    wcv_d = nc.dram_tensor("wcv", [128, 4, CB, 128], F32, kind="ExternalInput")
    wtt_d = nc.dram_tensor("wtt", [128, 2, 128], BF16, kind="ExternalInput")
    wch_d = nc.dram_tensor("wch", [128, 4, 128], F32, kind="ExternalInput")
    wct_d = nc.dram_tensor("wct", [128, CB, 128], BF16, kind="ExternalInput")
    bval_d = nc.dram_tensor("bval", [128, CB], F32, kind="ExternalInput")
    btdnn_d = nc.dram_tensor("btdnn", [128, 1], F32, kind="ExternalInput")
    bp_d = nc.dram_tensor("bp", [128, CB], F32, kind="ExternalInput")
    scal_d = nc.dram_tensor("scal", [128, 2], F32, kind="ExternalInput")
    out_d = nc.dram_tensor("out", [2 * C, 1], F32, kind="ExternalOutput")

    with tile.TileContext(nc) as tc:
        with (
            tc.tile_pool(name="consts", bufs=1) as cp,
            tc.tile_pool(name="xs", bufs=1) as xp,
            tc.tile_pool(name="hw", bufs=4) as hp,
            tc.tile_pool(name="pw", bufs=4) as pp,
            tc.tile_pool(name="vsbw", bufs=4) as vsp,
            tc.tile_pool(name="pvw", bufs=4) as pvp,
            tc.tile_pool(name="pv2w", bufs=2) as pv2p,
        ):
            # ---- load weights / consts ----
            wv1t = cp.tile([128, 2, CB, 128], BF16, tag="wv1t", name="wv1t")
            nc.sync.dma_start(out=wv1t, in_=wv1t_d[:, :, :, :])
            wcv = cp.tile([128, 4, CB, 128], F32, tag="wcv", name="wcv")
            nc.sync.dma_start(out=wcv, in_=wcv_d[:, :, :, :])
            wtt = cp.tile([128, 2, 128], BF16, tag="wtt", name="wtt")
            nc.sync.dma_start(out=wtt, in_=wtt_d[:, :, :])
            wch = cp.tile([128, 4, 128], F32, tag="wch", name="wch")
            nc.sync.dma_start(out=wch, in_=wch_d[:, :, :])
            wct = cp.tile([128, CB, 128], BF16, tag="wct", name="wct")
            nc.sync.dma_start(out=wct, in_=wct_d[:, :, :])
            bval = cp.tile([128, CB], F32, tag="bval", name="bval")
            nc.sync.dma_start(out=bval, in_=bval_d[:, :])
            btdnn = cp.tile([128, 1], F32, tag="btdnn", name="btdnn")
            nc.sync.dma_start(out=btdnn, in_=btdnn_d[:, :])
            bp = cp.tile([128, CB], F32, tag="bp", name="bp")
            nc.sync.dma_start(out=bp, in_=bp_d[:, :])
            scal = cp.tile([128, 2], F32, tag="scal", name="scal")
            nc.sync.dma_start(out=scal, in_=scal_d[:, :])

            # ---- load x (chunked) + stats accumulation (all on DVE) ----
            xs = []
            sumxp = []
            sumsqp = []
            dummy_a = cp.tile([128, LD], BF16, tag="dummy_a", name="dummy_a")
            for cb in range(CB):
                xs.append(xp.tile([128, L], BF16, tag=f"x{cb}", name=f"x{cb}"))
                sumxp.append(cp.tile([128, NDMA], F32, tag=f"sumxp{cb}", name=f"sumxp{cb}"))
                sumsqp.append(cp.tile([128, NDMA], F32, tag=f"sumsqp{cb}", name=f"sumsqp{cb}"))
            for cb in range(CB):
                for j in range(NDMA):
                    sl = slice(j * LD, (j + 1) * LD)
                    nc.sync.dma_start(
                        out=xs[cb][:, sl], in_=x_d[cb * 128 : (cb + 1) * 128, sl]
                    )
                    nc.vector.tensor_scalar(
                        out=dummy_a,
                        in0=xs[cb][:, sl],
                        scalar1=1.0,
                        scalar2=0.0,
                        op0=ALU.mult,
                        op1=ALU.add,
                        accum_out=sumxp[cb][:, j : j + 1],
                    )
                    nc.vector.scalar_tensor_tensor(
                        out=dummy_a,
                        in0=xs[cb][:, sl],
                        scalar=0.0,
                        in1=xs[cb][:, sl],
                        op0=ALU.bypass,
                        op1=ALU.mult,
                        accum_out=sumsqp[cb][:, j : j + 1],
                    )

            # ---- finalize stats: gmean / gstd per c-block ----
            gmean, gstd = [], []
            for cb in range(CB):
                sx = cp.tile([128, 1], F32, tag=f"sx{cb}", name=f"sx{cb}")
                nc.vector.tensor_reduce(
                    out=sx, in_=sumxp[cb], axis=mybir.AxisListType.X, op=ALU.add
                )
                sq = cp.tile([128, 1], F32, tag=f"sq{cb}", name=f"sq{cb}")
                nc.vector.tensor_reduce(
                    out=sq, in_=sumsqp[cb], axis=mybir.AxisListType.X, op=ALU.add
                )
                gm = cp.tile([128, 1], F32, tag=f"gm{cb}", name=f"gm{cb}")
                nc.vector.tensor_scalar_mul(out=gm, in0=sx, scalar1=scal[:, 0:1])
                msq = cp.tile([128, 1], F32, tag=f"msq{cb}", name=f"msq{cb}")
                nc.vector.tensor_scalar_mul(out=msq, in0=sq, scalar1=scal[:, 0:1])
                gm2 = cp.tile([128, 1], F32, tag=f"gm2{cb}", name=f"gm2{cb}")
                nc.vector.tensor_mul(out=gm2, in0=gm, in1=gm)
                gv = cp.tile([128, 1], F32, tag=f"gv{cb}", name=f"gv{cb}")
                nc.vector.tensor_sub(out=gv, in0=msq, in1=gm2)
                nc.vector.tensor_scalar_max(out=gv, in0=gv, scalar1=EPS)
                # gstd = exp(0.5*ln(gv)) -- keeps ACT on one table set (ln+exp)
                lng = cp.tile([128, 1], F32, tag=f"lng{cb}", name=f"lng{cb}")
                nc.scalar.activation(out=lng, in_=gv, func=ACT.Ln)
                gs = cp.tile([128, 1], F32, tag=f"gs{cb}", name=f"gs{cb}")
                nc.scalar.activation(out=gs, in_=lng, func=ACT.Exp, scale=0.5)
                gmean.append(gm)
                gstd.append(gs)

            gg = [gmean[0], gmean[1], gstd[0], gstd[1]]  # [f'] blocks of [gmean; gstd]

            # ---- derived consts via tiny PE matvecs (fp32) ----
            with tc.tile_pool(name="pscst", bufs=2, space="PSUM") as pcst:
                ps_ch = pcst.tile([128, 1], F32, tag="pch", name="pch")
                for j in range(4):
                    nc.tensor.matmul(
                        ps_ch, lhsT=wch[:, j, :], rhs=gg[j], start=(j == 0), stop=(j == 3)
                    )
                const_h = cp.tile([128, 1], F32, tag="const_h", name="const_h")
                nc.scalar.activation(
                    out=const_h, in_=ps_ch, func=ACT.Identity, bias=btdnn[:, 0:1]
                )
                hinv = cp.tile([128, 1], BF16, tag="hinv", name="hinv")
                nc.scalar.activation(out=hinv, in_=const_h, func=ACT.Relu)

                cv, pinv = [], []
                for cb in range(CB):
                    ps_cv = pcst.tile([128, 1], F32, tag="pcv", name="pcv")
                    for j in range(4):
                        nc.tensor.matmul(
                            ps_cv,
                            lhsT=wcv[:, j, cb, :],
                            rhs=gg[j],
                            start=(j == 0),
                            stop=(j == 3),
                        )
                    cvt = cp.tile([128, 1], F32, tag=f"cv{cb}", name=f"cv{cb}")
                    nc.scalar.activation(
                        out=cvt, in_=ps_cv, func=ACT.Identity, bias=bval[:, cb : cb + 1]
                    )
                    cv.append(cvt)
                    ps_si = pcst.tile([128, 1], F32, tag="psi", name="psi")
                    nc.tensor.matmul(ps_si, lhsT=wct[:, cb, :], rhs=hinv, start=True, stop=True)
                    pv_t = cp.tile([128, 1], F32, tag=f"pinv{cb}", name=f"pinv{cb}")
                    nc.scalar.activation(
                        out=pv_t, in_=ps_si, func=ACT.Exp, bias=bp[:, cb : cb + 1]
                    )
                    pinv.append(pv_t)

            # ---- streaming phase ----
            Zp = [cp.tile([128, NCHUNK], F32, tag=f"Zp{cb}", name=f"Zp{cb}") for cb in range(CB)]
            S1p = [cp.tile([128, NCHUNK], F32, tag=f"S1p{cb}", name=f"S1p{cb}") for cb in range(CB)]
            S2p = [cp.tile([128, NCHUNK], F32, tag=f"S2p{cb}", name=f"S2p{cb}") for cb in range(CB)]

            with (
                tc.tile_pool(name="psh", bufs=2, space="PSUM") as ps_h,
                tc.tile_pool(name="psv", bufs=4, space="PSUM") as ps_v,
                tc.tile_pool(name="pss", bufs=2, space="PSUM") as ps_s,
            ):
                for i in range(NCHUNK):
                    sl = slice(i * LC, (i + 1) * LC)
                    ph = ps_h.tile([128, LC], F32, tag="ph", name="ph")
                    nc.tensor.matmul(ph, lhsT=wtt[:, 0, :], rhs=xs[0][:, sl], start=True, stop=False)
                    nc.tensor.matmul(ph, lhsT=wtt[:, 1, :], rhs=xs[1][:, sl], start=False, stop=True)
                    h = hp.tile([128, LC], BF16, tag="h", name="h")
                    nc.scalar.activation(out=h, in_=ph, func=ACT.Relu, bias=const_h[:, 0:1])
                    for cb in range(CB):
                        vps = ps_v.tile([128, LC], F32, tag="vraw", name="vraw")
                        nc.tensor.matmul(vps, lhsT=wv1t[:, 0, cb, :], rhs=xs[0][:, sl], start=True, stop=False)
                        nc.tensor.matmul(vps, lhsT=wv1t[:, 1, cb, :], rhs=xs[1][:, sl], start=False, stop=True)
                        vsb = vsp.tile([128, LC], BF16, tag="vsb", name="vsb")
                        nc.vector.tensor_copy(out=vsb, in_=vps)
                        sps = ps_s.tile([128, LC], F32, tag="scr", name="scr")
                        nc.tensor.matmul(sps, lhsT=wct[:, cb, :], rhs=h, start=True, stop=True)
                        p = pp.tile([128, LC], BF16, tag="p", name="p")
                        nc.scalar.activation(
                            out=p, in_=sps, func=ACT.Exp,
                            bias=bp[:, cb : cb + 1],
                            accum_out=Zp[cb][:, i : i + 1],
                        )
                        pv = pvp.tile([128, LC], BF16, tag="pv", name="pv")
                        nc.vector.scalar_tensor_tensor(
                            out=pv, in0=p, scalar=0.0, in1=vsb,
                            op0=ALU.bypass, op1=ALU.mult,
                            accum_out=S1p[cb][:, i : i + 1],
                        )
                        pv2 = pv2p.tile([128, LC], BF16, tag="pv2", name="pv2")
                        nc.vector.scalar_tensor_tensor(
                            out=pv2, in0=pv, scalar=0.0, in1=vsb,
                            op0=ALU.bypass, op1=ALU.mult,
                            accum_out=S2p[cb][:, i : i + 1],
                        )

            # ---- finalize ----
            for cb in range(CB):
                Z = cp.tile([128, 1], F32, tag=f"Z{cb}", name=f"Z{cb}")
                nc.vector.tensor_reduce(out=Z, in_=Zp[cb], axis=mybir.AxisListType.X, op=ALU.add)
                corr = cp.tile([128, 1], F32, tag=f"corr{cb}", name=f"corr{cb}")
                nc.vector.tensor_scalar_mul(out=corr, in0=pinv[cb], scalar1=scal[:, 1:2])
                Zv = cp.tile([128, 1], F32, tag=f"Zv{cb}", name=f"Zv{cb}")
                nc.vector.tensor_sub(out=Zv, in0=Z, in1=corr)
                rz = cp.tile([128, 1], F32, tag=f"rz{cb}", name=f"rz{cb}")
                nc.vector.reciprocal(out=rz, in_=Zv)
                S1 = cp.tile([128, 1], F32, tag=f"S1{cb}", name=f"S1{cb}")
                nc.vector.tensor_reduce(out=S1, in_=S1p[cb], axis=mybir.AxisListType.X, op=ALU.add)
                S2 = cp.tile([128, 1], F32, tag=f"S2{cb}", name=f"S2{cb}")
                nc.vector.tensor_reduce(out=S2, in_=S2p[cb], axis=mybir.AxisListType.X, op=ALU.add)
                m1 = cp.tile([128, 1], F32, tag=f"m1{cb}", name=f"m1{cb}")
                nc.vector.tensor_scalar_mul(out=m1, in0=S1, scalar1=rz)
                amean = cp.tile([128, 1], F32, tag=f"amean{cb}", name=f"amean{cb}")
                nc.vector.tensor_add(out=amean, in0=m1, in1=cv[cb])
                t1 = cp.tile([128, 1], F32, tag=f"t1{cb}", name=f"t1{cb}")
                nc.vector.tensor_scalar_mul(out=t1, in0=S2, scalar1=rz)
                m1sq = cp.tile([128, 1], F32, tag=f"m1sq{cb}", name=f"m1sq{cb}")
                nc.vector.tensor_mul(out=m1sq, in0=m1, in1=m1)
                avar = cp.tile([128, 1], F32, tag=f"avar{cb}", name=f"avar{cb}")
                nc.vector.tensor_sub(out=avar, in0=t1, in1=m1sq)
                nc.vector.tensor_scalar_max(out=avar, in0=avar, scalar1=EPS)
                lnv = cp.tile([128, 1], F32, tag=f"lnv{cb}", name=f"lnv{cb}")
                nc.scalar.activation(out=lnv, in_=avar, func=ACT.Ln)
                astd = cp.tile([128, 1], F32, tag=f"astd{cb}", name=f"astd{cb}")
                nc.scalar.activation(out=astd, in_=lnv, func=ACT.Exp, scale=0.5)
                nc.sync.dma_start(out=out_d[cb * 128 : (cb + 1) * 128, :], in_=amean)
                nc.sync.dma_start(out=out_d[C + cb * 128 : C + (cb + 1) * 128, :], in_=astd)

    _split_multiwaits(nc)
    return nc


_NC_CACHE = None


def _get_nc():
    global _NC_CACHE
    if _NC_CACHE is None:
        _NC_CACHE = _build_nc()
    return _NC_CACHE


def _prep_inputs(x, lengths, w_val, b_val, w_tdnn, b_tdnn, bn_gamma, bn_beta,
                 w_conv, b_conv):
    x = np.asarray(x, dtype=np.float32)
    lengths = np.asarray(lengths, dtype=np.float32)
    w_val = np.asarray(w_val, dtype=np.float32)
    b_val = np.asarray(b_val, dtype=np.float32)
    w_tdnn = np.asarray(w_tdnn, dtype=np.float32)
    b_tdnn = np.asarray(b_tdnn, dtype=np.float32)
    bn_gamma = np.asarray(bn_gamma, dtype=np.float32)
    bn_beta = np.asarray(bn_beta, dtype=np.float32)
    w_conv = np.asarray(w_conv, dtype=np.float32)
    b_conv = np.asarray(b_conv, dtype=np.float32)

    mask = (np.arange(L, dtype=np.float32)[None, :] < (lengths * L)[:, None])
    total = mask.sum(axis=1).astype(np.float32)            # [B]
    xm = (x * mask[:, None, :].astype(np.float32)).astype(ml_dtypes.bfloat16)

    def pack_lhsT(w, kblocks, cblocks, dt=None):
        # w: [K, M] (contraction-major) -> [128, kblocks, cblocks, 128]
        Ktot, Mtot = w.shape
        assert Ktot == kblocks * 128 and Mtot == cblocks * 128
        r = np.ascontiguousarray(
            w.reshape(kblocks, 128, cblocks, 128).transpose(1, 0, 2, 3)
        )
        return r.astype(dt) if dt is not None else r

    W1T = w_val[:, :C].T                                   # [f, c]
    wv1t = pack_lhsT(W1T, 2, CB, ml_dtypes.bfloat16)
    Wcv = np.concatenate([w_val[:, C:2 * C].T, w_val[:, 2 * C:].T], axis=0)  # [2C, C]
    wcv = pack_lhsT(Wcv, 4, CB)
    WtT = w_tdnn[:, :C].T                                  # [f, a]
    wtt = pack_lhsT(WtT, 2, 1, ml_dtypes.bfloat16).reshape(128, 2, 128)
    Wch = np.concatenate([w_tdnn[:, C:2 * C].T, w_tdnn[:, 2 * C:].T], axis=0)
    wch = pack_lhsT(Wch, 4, 1).reshape(128, 4, 128)
    WcT = (w_conv * bn_gamma[None, :]).T                   # [a, c]
    wct = pack_lhsT(WcT, 1, CB, ml_dtypes.bfloat16).reshape(128, CB, 128)
    bprime = b_conv + w_conv @ bn_beta                     # [C]

    shared = {
        "wv1t": wv1t, "wcv": wcv, "wtt": wtt, "wch": wch, "wct": wct,
        "bval": np.ascontiguousarray(b_val.reshape(CB, 128).T),
        "btdnn": np.ascontiguousarray(b_tdnn.reshape(128, 1)),
        "bp": np.ascontiguousarray(bprime.reshape(CB, 128).T),
    }
    in_maps = []
    for b in range(B):
        m = dict(shared)
        m["x"] = np.ascontiguousarray(xm[b])
        scal = np.empty((128, 2), dtype=np.float32)
        scal[:, 0] = 1.0 / total[b]
        scal[:, 1] = L - total[b]
        m["scal"] = scal
        in_maps.append(m)
    return in_maps


def kernel(**inputs) -> np.ndarray:
    in_maps = _prep_inputs(**inputs)
    nc = _get_nc()
    res = run_bass_kernel_spmd(nc, in_maps, core_ids=list(range(B)))
    out = np.stack([res.results[b]["out"] for b in range(B)], axis=0)  # [B, 2C, 1]
    return out.astype(np.float32)


# revision 5
# speedup vs baseline: 1.7753x; 1.2945x over previous
"""Trainium2 Bass kernel for ContinuousAttentiveStatisticsPooling.

Shape config (hardcoded): B=8, C=256, L=8192, A=128, 8 NeuronCores,
pure data parallel over B (one example per core).

Math restructure (per example, x is [C, L]):
  - Host zeroes x beyond the valid length -> all L-reductions over full L
    equal masked reductions (gmean/gstd; and W @ x has exact-zero tails).
  - gmean = sum(x)/total ; gstd = sqrt(clip(sum(x^2)/total - gmean^2))
  - vraw     = W1 @ x  (no bias; amean/avar reconstructed from raw moments)
  - pre_h    = Wt1 @ x + ch,  ch = Wt2 @ gmean + Wt3 @ gstd + b_tdnn
  - h        = relu(pre_h)       (gamma folded into Wc', beta into b')
  - scores   = Wc' @ h + b'
  - p        = exp(scores)   (no max subtraction; scores are O(1))
  - The invalid tail of scores is an exactly-computable constant s_inv, so
    Z_valid = sum_L p - n_invalid * exp(s_inv + b') ; p*vraw has zero tail.
  - amean = S1/Z + cv ; avar = S2/Z - (S1/Z)^2 with
    S1 = sum p*vraw, S2 = sum p*vraw^2, cv = W2@gmean + W3@gstd + b_val.

Precision: x / matmul weights / elementwise interiors in bf16; all
reductions, biases, and the derived consts in fp32.
"""

import sys

if "/opt/trn_rl_repo" not in sys.path:
    sys.path.insert(0, "/opt/trn_rl_repo")

import numpy as np
import ml_dtypes

import concourse.bass as bass
import concourse.mybir as mybir
import concourse.tile as tile
from concourse.bass_utils import run_bass_kernel_spmd

B, C, L, A = 8, 256, 8192, 128
CB = C // 128          # 2 c-blocks
NCHUNK = 16            # streaming chunks over L
LC = L // NCHUNK       # 512
NDMA = 4               # x DMA chunks per c-block
LD = L // NDMA         # 2048
EPS = 1e-12
F32 = mybir.dt.float32
BF16 = mybir.dt.bfloat16
ALU = mybir.AluOpType
ACT = mybir.ActivationFunctionType

_mw_ctr = [0]


def _split_multiwaits(nc):
    """This walrus build supports only ONE sync-wait per instruction.
    Split multi-wait instructions into single-wait NoOps on the same engine
    (same-engine program order preserves semantics exactly)."""
    for f in nc.m.functions:
        for blk in f.blocks:
            insts = blk.instructions
            out = []
            changed = False
            for inst in insts:
                si = inst.sync_info
                if si is not None and len(si.on_wait) > 1:
                    changed = True
                    waits = list(si.on_wait)
                    for w in waits[:-1]:
                        _mw_ctr[0] += 1
                        nop = mybir.InstNoOp(
                            name=f"mwsplit-{_mw_ctr[0]}", ins=[], outs=[]
                        )
                        nop.engine = inst.engine
                        nop.sync_info = mybir.SyncInfo(on_wait=[w], on_update=[])
                        out.append(nop)
                    inst.sync_info = mybir.SyncInfo(
                        on_wait=[waits[-1]], on_update=list(si.on_update)
                    )
                out.append(inst)
            if changed:
                insts[:] = out


def _build_nc():
    nc = bass.Bass()
    x_d = nc.dram_tensor("x", [C, L], BF16, kind="ExternalInput")
    wv1t_d = nc.dram_tensor("wv1t", [128, 2, CB, 128], BF16, kind="ExternalInput")
    wcv_d = nc.dram_tensor("wcv", [128, 4, CB, 128], F32, kind="ExternalInput")
    wtt_d = nc.dram_tensor("wtt", [128, 2, 128], BF16, kind="ExternalInput")
    wch_d = nc.dram_tensor("wch", [128, 4, 128], F32, kind="ExternalInput")
    wct_d = nc.dram_tensor("wct", [128, CB, 128], BF16, kind="ExternalInput")
    bval_d = nc.dram_tensor("bval", [128, CB], F32, kind="ExternalInput")
    btdnn_d = nc.dram_tensor("btdnn", [128, 1], F32, kind="ExternalInput")
    bp_d = nc.dram_tensor("bp", [128, CB], F32, kind="ExternalInput")
    scal_d = nc.dram_tensor("scal", [128, 2], F32, kind="ExternalInput")
    out_d = nc.dram_tensor("out", [2 * C, 1], F32, kind="ExternalOutput")

    with tile.TileContext(nc) as tc:
        with (
            tc.tile_pool(name="consts", bufs=1) as cp,
            tc.tile_pool(name="xs", bufs=1) as xp,
            tc.tile_pool(name="hw", bufs=4) as hp,
            tc.tile_pool(name="pw", bufs=4) as pp,
            tc.tile_pool(name="pvw", bufs=4) as pvp,
            tc.tile_pool(name="pv2w", bufs=2) as pv2p,
        ):
            # ---- load weights / consts ----
            wv1t = cp.tile([128, 2, CB, 128], BF16, tag="wv1t", name="wv1t")
            nc.sync.dma_start(out=wv1t, in_=wv1t_d[:, :, :, :])
            wcv = cp.tile([128, 4, CB, 128], F32, tag="wcv", name="wcv")
            nc.sync.dma_start(out=wcv, in_=wcv_d[:, :, :, :])
            wtt = cp.tile([128, 2, 128], BF16, tag="wtt", name="wtt")
            nc.sync.dma_start(out=wtt, in_=wtt_d[:, :, :])
            wch = cp.tile([128, 4, 128], F32, tag="wch", name="wch")
            nc.sync.dma_start(out=wch, in_=wch_d[:, :, :])
            wct = cp.tile([128, CB, 128], BF16, tag="wct", name="wct")
            nc.sync.dma_start(out=wct, in_=wct_d[:, :, :])
            bval = cp.tile([128, CB], F32, tag="bval", name="bval")
            nc.sync.dma_start(out=bval, in_=bval_d[:, :])
            btdnn = cp.tile([128, 1], F32, tag="btdnn", name="btdnn")
            nc.sync.dma_start(out=btdnn, in_=btdnn_d[:, :])
            bp = cp.tile([128, CB], F32, tag="bp", name="bp")
            nc.sync.dma_start(out=bp, in_=bp_d[:, :])
            scal = cp.tile([128, 2], F32, tag="scal", name="scal")
            nc.sync.dma_start(out=scal, in_=scal_d[:, :])

            # ---- load x (chunked) + stats accumulation (all on DVE) ----
            xs = []
            sumxp = []
            sumsqp = []
            dummy_a = cp.tile([128, LD], BF16, tag="dummy_a", name="dummy_a")
            dummy_b = cp.tile([128, LD], BF16, tag="dummy_b", name="dummy_b")
            for cb in range(CB):
                xs.append(xp.tile([128, L], BF16, tag=f"x{cb}", name=f"x{cb}"))
                sumxp.append(cp.tile([128, NDMA], F32, tag=f"sumxp{cb}", name=f"sumxp{cb}"))
                sumsqp.append(cp.tile([128, NDMA], F32, tag=f"sumsqp{cb}", name=f"sumsqp{cb}"))
            for cb in range(CB):
                for j in range(NDMA):
                    sl = slice(j * LD, (j + 1) * LD)
                    nc.sync.dma_start(
                        out=xs[cb][:, sl], in_=x_d[cb * 128 : (cb + 1) * 128, sl]
                    )
                    nc.vector.tensor_scalar(
                        out=dummy_a,
                        in0=xs[cb][:, sl],
                        scalar1=1.0,
                        scalar2=0.0,
                        op0=ALU.mult,
                        op1=ALU.add,
                        accum_out=sumxp[cb][:, j : j + 1],
                    )
                    nc.scalar.activation(
                        out=dummy_b,
                        in_=xs[cb][:, sl],
                        func=ACT.Square,
                        accum_out=sumsqp[cb][:, j : j + 1],
                    )

            # ---- finalize stats: gmean / gstd per c-block ----
            gmean, gstd = [], []
            for cb in range(CB):
                sx = cp.tile([128, 1], F32, tag=f"sx{cb}", name=f"sx{cb}")
                nc.vector.tensor_reduce(
                    out=sx, in_=sumxp[cb], axis=mybir.AxisListType.X, op=ALU.add
                )
                sq = cp.tile([128, 1], F32, tag=f"sq{cb}", name=f"sq{cb}")
                nc.vector.tensor_reduce(
                    out=sq, in_=sumsqp[cb], axis=mybir.AxisListType.X, op=ALU.add
                )
                gm = cp.tile([128, 1], F32, tag=f"gm{cb}", name=f"gm{cb}")
                nc.vector.tensor_scalar_mul(out=gm, in0=sx, scalar1=scal[:, 0:1])
                msq = cp.tile([128, 1], F32, tag=f"msq{cb}", name=f"msq{cb}")
                nc.vector.tensor_scalar_mul(out=msq, in0=sq, scalar1=scal[:, 0:1])
                gm2 = cp.tile([128, 1], F32, tag=f"gm2{cb}", name=f"gm2{cb}")
                nc.vector.tensor_mul(out=gm2, in0=gm, in1=gm)
                gv = cp.tile([128, 1], F32, tag=f"gv{cb}", name=f"gv{cb}")
                nc.vector.tensor_sub(out=gv, in0=msq, in1=gm2)
                nc.vector.tensor_scalar_max(out=gv, in0=gv, scalar1=EPS)
                # gstd = exp(0.5*ln(gv)) -- keeps ACT on one table set (ln+exp)
                lng = cp.tile([128, 1], F32, tag=f"lng{cb}", name=f"lng{cb}")
                nc.scalar.activation(out=lng, in_=gv, func=ACT.Ln)
                gs = cp.tile([128, 1], F32, tag=f"gs{cb}", name=f"gs{cb}")
                nc.scalar.activation(out=gs, in_=lng, func=ACT.Exp, scale=0.5)
                gmean.append(gm)
                gstd.append(gs)

            gg = [gmean[0], gmean[1], gstd[0], gstd[1]]  # [f'] blocks of [gmean; gstd]

            # ---- derived consts via tiny PE matvecs (fp32) ----
            with tc.tile_pool(name="pscst", bufs=2, space="PSUM") as pcst:
                ps_ch = pcst.tile([128, 1], F32, tag="pch", name="pch")
                for j in range(4):
                    nc.tensor.matmul(
                        ps_ch, lhsT=wch[:, j, :], rhs=gg[j], start=(j == 0), stop=(j == 3)
                    )
                const_h = cp.tile([128, 1], F32, tag="const_h", name="const_h")
                nc.scalar.activation(
                    out=const_h, in_=ps_ch, func=ACT.Identity, bias=btdnn[:, 0:1]
                )
                hinv = cp.tile([128, 1], BF16, tag="hinv", name="hinv")
                nc.scalar.activation(out=hinv, in_=const_h, func=ACT.Relu)

                cv, pinv = [], []
                for cb in range(CB):
                    ps_cv = pcst.tile([128, 1], F32, tag="pcv", name="pcv")
                    for j in range(4):
                        nc.tensor.matmul(
                            ps_cv,
                            lhsT=wcv[:, j, cb, :],
                            rhs=gg[j],
                            start=(j == 0),
                            stop=(j == 3),
                        )
                    cvt = cp.tile([128, 1], F32, tag=f"cv{cb}", name=f"cv{cb}")
                    nc.scalar.activation(
                        out=cvt, in_=ps_cv, func=ACT.Identity, bias=bval[:, cb : cb + 1]
                    )
                    cv.append(cvt)
                    ps_si = pcst.tile([128, 1], F32, tag="psi", name="psi")
                    nc.tensor.matmul(ps_si, lhsT=wct[:, cb, :], rhs=hinv, start=True, stop=True)
                    pv_t = cp.tile([128, 1], F32, tag=f"pinv{cb}", name=f"pinv{cb}")
                    nc.scalar.activation(
                        out=pv_t, in_=ps_si, func=ACT.Exp, bias=bp[:, cb : cb + 1]
                    )
                    pinv.append(pv_t)

            # ---- streaming phase ----
            Zp = [cp.tile([128, NCHUNK], F32, tag=f"Zp{cb}", name=f"Zp{cb}") for cb in range(CB)]
            S1p = [cp.tile([128, NCHUNK], F32, tag=f"S1p{cb}", name=f"S1p{cb}") for cb in range(CB)]
            S2p = [cp.tile([128, NCHUNK], F32, tag=f"S2p{cb}", name=f"S2p{cb}") for cb in range(CB)]

            with (
                tc.tile_pool(name="psh", bufs=2, space="PSUM") as ps_h,
                tc.tile_pool(name="psv", bufs=4, space="PSUM") as ps_v,
                tc.tile_pool(name="pss", bufs=2, space="PSUM") as ps_s,
            ):
                for i in range(NCHUNK):
                    sl = slice(i * LC, (i + 1) * LC)
                    ph = ps_h.tile([128, LC], F32, tag="ph", name="ph")
                    nc.tensor.matmul(ph, lhsT=wtt[:, 0, :], rhs=xs[0][:, sl], start=True, stop=False)
                    nc.tensor.matmul(ph, lhsT=wtt[:, 1, :], rhs=xs[1][:, sl], start=False, stop=True)
                    h = hp.tile([128, LC], BF16, tag="h", name="h")
                    nc.scalar.activation(out=h, in_=ph, func=ACT.Relu, bias=const_h[:, 0:1])
                    for cb in range(CB):
                        vps = ps_v.tile([128, LC], F32, tag="vraw", name="vraw")
                        nc.tensor.matmul(vps, lhsT=wv1t[:, 0, cb, :], rhs=xs[0][:, sl], start=True, stop=False)
                        nc.tensor.matmul(vps, lhsT=wv1t[:, 1, cb, :], rhs=xs[1][:, sl], start=False, stop=True)
                        sps = ps_s.tile([128, LC], F32, tag="scr", name="scr")
                        nc.tensor.matmul(sps, lhsT=wct[:, cb, :], rhs=h, start=True, stop=True)
                        p = pp.tile([128, LC], BF16, tag="p", name="p")
                        nc.scalar.activation(
                            out=p, in_=sps, func=ACT.Exp,
                            bias=bp[:, cb : cb + 1],
                            accum_out=Zp[cb][:, i : i + 1],
                        )
                        pv = pvp.tile([128, LC], BF16, tag="pv", name="pv")
                        nc.vector.scalar_tensor_tensor(
                            out=pv, in0=p, scalar=0.0, in1=vps,
                            op0=ALU.bypass, op1=ALU.mult,
                            accum_out=S1p[cb][:, i : i + 1],
                        )
                        pv2 = pv2p.tile([128, LC], BF16, tag="pv2", name="pv2")
                        nc.vector.scalar_tensor_tensor(
                            out=pv2, in0=pv, scalar=0.0, in1=vps,
                            op0=ALU.bypass, op1=ALU.mult,
                            accum_out=S2p[cb][:, i : i + 1],
                        )

            # ---- finalize ----
            for cb in range(CB):
                Z = cp.tile([128, 1], F32, tag=f"Z{cb}", name=f"Z{cb}")
                nc.vector.tensor_reduce(out=Z, in_=Zp[cb], axis=mybir.AxisListType.X, op=ALU.add)
                corr = cp.tile([128, 1], F32, tag=f"corr{cb}", name=f"corr{cb}")
                nc.vector.tensor_scalar_mul(out=corr, in0=pinv[cb], scalar1=scal[:, 1:2])
                Zv = cp.tile([128, 1], F32, tag=f"Zv{cb}", name=f"Zv{cb}")
                nc.vector.tensor_sub(out=Zv, in0=Z, in1=corr)
                rz = cp.tile([128, 1], F32, tag=f"rz{cb}", name=f"rz{cb}")
                nc.vector.reciprocal(out=rz, in_=Zv)
                S1 = cp.tile([128, 1], F32, tag=f"S1{cb}", name=f"S1{cb}")
                nc.vector.tensor_reduce(out=S1, in_=S1p[cb], axis=mybir.AxisListType.X, op=ALU.add)
                S2 = cp.tile([128, 1], F32, tag=f"S2{cb}", name=f"S2{cb}")
                nc.vector.tensor_reduce(out=S2, in_=S2p[cb], axis=mybir.AxisListType.X, op=ALU.add)
                m1 = cp.tile([128, 1], F32, tag=f"m1{cb}", name=f"m1{cb}")
                nc.vector.tensor_scalar_mul(out=m1, in0=S1, scalar1=rz)
                amean = cp.tile([128, 1], F32, tag=f"amean{cb}", name=f"amean{cb}")
                nc.vector.tensor_add(out=amean, in0=m1, in1=cv[cb])
                t1 = cp.tile([128, 1], F32, tag=f"t1{cb}", name=f"t1{cb}")
                nc.vector.tensor_scalar_mul(out=t1, in0=S2, scalar1=rz)
                m1sq = cp.tile([128, 1], F32, tag=f"m1sq{cb}", name=f"m1sq{cb}")
                nc.vector.tensor_mul(out=m1sq, in0=m1, in1=m1)
                avar = cp.tile([128, 1], F32, tag=f"avar{cb}", name=f"avar{cb}")
                nc.vector.tensor_sub(out=avar, in0=t1, in1=m1sq)
                nc.vector.tensor_scalar_max(out=avar, in0=avar, scalar1=EPS)
                lnv = cp.tile([128, 1], F32, tag=f"lnv{cb}", name=f"lnv{cb}")
                nc.scalar.activation(out=lnv, in_=avar, func=ACT.Ln)
                astd = cp.tile([128, 1], F32, tag=f"astd{cb}", name=f"astd{cb}")
                nc.scalar.activation(out=astd, in_=lnv, func=ACT.Exp, scale=0.5)
                nc.sync.dma_start(out=out_d[cb * 128 : (cb + 1) * 128, :], in_=amean)
                nc.sync.dma_start(out=out_d[C + cb * 128 : C + (cb + 1) * 128, :], in_=astd)

    _split_multiwaits(nc)
    return nc


_NC_CACHE = None


def _get_nc():
    global _NC_CACHE
    if _NC_CACHE is None:
        _NC_CACHE = _build_nc()
    return _NC_CACHE


def _prep_inputs(x, lengths, w_val, b_val, w_tdnn, b_tdnn, bn_gamma, bn_beta,
                 w_conv, b_conv):
    x = np.asarray(x, dtype=np.float32)
    lengths = np.asarray(lengths, dtype=np.float32)
    w_val = np.asarray(w_val, dtype=np.float32)
    b_val = np.asarray(b_val, dtype=np.float32)
    w_tdnn = np.asarray(w_tdnn, dtype=np.float32)
    b_tdnn = np.asarray(b_tdnn, dtype=np.float32)
    bn_gamma = np.asarray(bn_gamma, dtype=np.float32)
    bn_beta = np.asarray(bn_beta, dtype=np.float32)
    w_conv = np.asarray(w_conv, dtype=np.float32)
    b_conv = np.asarray(b_conv, dtype=np.float32)

    mask = (np.arange(L, dtype=np.float32)[None, :] < (lengths * L)[:, None])
    total = mask.sum(axis=1).astype(np.float32)            # [B]
    xm = (x * mask[:, None, :].astype(np.float32)).astype(ml_dtypes.bfloat16)

    def pack_lhsT(w, kblocks, cblocks, dt=None):
        # w: [K, M] (contraction-major) -> [128, kblocks, cblocks, 128]
        Ktot, Mtot = w.shape
        assert Ktot == kblocks * 128 and Mtot == cblocks * 128
        r = np.ascontiguousarray(
            w.reshape(kblocks, 128, cblocks, 128).transpose(1, 0, 2, 3)
        )
        return r.astype(dt) if dt is not None else r

    W1T = w_val[:, :C].T                                   # [f, c]
    wv1t = pack_lhsT(W1T, 2, CB, ml_dtypes.bfloat16)
    Wcv = np.concatenate([w_val[:, C:2 * C].T, w_val[:, 2 * C:].T], axis=0)  # [2C, C]
    wcv = pack_lhsT(Wcv, 4, CB)
    WtT = w_tdnn[:, :C].T                                  # [f, a]
    wtt = pack_lhsT(WtT, 2, 1, ml_dtypes.bfloat16).reshape(128, 2, 128)
    Wch = np.concatenate([w_tdnn[:, C:2 * C].T, w_tdnn[:, 2 * C:].T], axis=0)
    wch = pack_lhsT(Wch, 4, 1).reshape(128, 4, 128)
    WcT = (w_conv * bn_gamma[None, :]).T                   # [a, c]
    wct = pack_lhsT(WcT, 1, CB, ml_dtypes.bfloat16).reshape(128, CB, 128)
    bprime = b_conv + w_conv @ bn_beta                     # [C]

    shared = {
        "wv1t": wv1t, "wcv": wcv, "wtt": wtt, "wch": wch, "wct": wct,
        "bval": np.ascontiguousarray(b_val.reshape(CB, 128).T),
        "btdnn": np.ascontiguousarray(b_tdnn.reshape(128, 1)),
        "bp": np.ascontiguousarray(bprime.reshape(CB, 128).T),
    }
    in_maps = []
    for b in range(B):
        m = dict(shared)
        m["x"] = np.ascontiguousarray(xm[b])
        scal = np.empty((128, 2), dtype=np.float32)
        scal[:, 0] = 1.0 / total[b]
        scal[:, 1] = L - total[b]
        m["scal"] = scal
        in_maps.append(m)
    return in_maps


def kernel(**inputs) -> np.ndarray:
    in_maps = _prep_inputs(**inputs)
    nc = _get_nc()
    res = run_bass_kernel_spmd(nc, in_maps, core_ids=list(range(B)))
    out = np.stack([res.results[b]["out"] for b in range(B)], axis=0)  # [B, 2C, 1]
    return out.astype(np.float32)


# revision 6
# speedup vs baseline: 1.7769x; 1.0009x over previous
"""Trainium2 Bass kernel for ContinuousAttentiveStatisticsPooling.

Shape config (hardcoded): B=8, C=256, L=8192, A=128, 8 NeuronCores,
pure data parallel over B (one example per core).

Math restructure (per example, x is [C, L]):
  - Host zeroes x beyond the valid length -> all L-reductions over full L
    equal masked reductions (gmean/gstd; and W @ x has exact-zero tails).
  - gmean = sum(x)/total ; gstd = sqrt(clip(sum(x^2)/total - gmean^2))
  - vraw     = W1 @ x  (no bias; amean/avar reconstructed from raw moments)
  - pre_h    = Wt1 @ x + ch,  ch = Wt2 @ gmean + Wt3 @ gstd + b_tdnn
  - h        = relu(pre_h)       (gamma folded into Wc', beta into b')
  - scores   = Wc' @ h + b'
  - p        = exp(scores)   (no max subtraction; scores are O(1))
  - The invalid tail of scores is an exactly-computable constant s_inv, so
    Z_valid = sum_L p - n_invalid * exp(s_inv + b') ; p*vraw has zero tail.
  - amean = S1/Z + cv ; avar = S2/Z - (S1/Z)^2 with
    S1 = sum p*vraw, S2 = sum p*vraw^2, cv = W2@gmean + W3@gstd + b_val.

Precision: x / matmul weights / elementwise interiors in bf16; all
reductions, biases, and the derived consts in fp32.
"""

import sys

if "/opt/trn_rl_repo" not in sys.path:
    sys.path.insert(0, "/opt/trn_rl_repo")

import numpy as np
import ml_dtypes

import concourse.bass as bass
import concourse.mybir as mybir
import concourse.tile as tile
from concourse.bass_utils import run_bass_kernel_spmd

B, C, L, A = 8, 256, 8192, 128
CB = C // 128          # 2 c-blocks
NCHUNK = 16            # streaming chunks over L
LC = L // NCHUNK       # 512
NDMA = 4               # x DMA chunks per c-block
LD = L // NDMA         # 2048
EPS = 1e-12
F32 = mybir.dt.float32
BF16 = mybir.dt.bfloat16
ALU = mybir.AluOpType
ACT = mybir.ActivationFunctionType

_mw_ctr = [0]


def _split_multiwaits(nc):
    """This walrus build supports only ONE sync-wait per instruction.
    Split multi-wait instructions into single-wait NoOps on the same engine
    (same-engine program order preserves semantics exactly)."""
    for f in nc.m.functions:
        for blk in f.blocks:
            insts = blk.instructions
            out = []
            changed = False
            for inst in insts:
                si = inst.sync_info
                if si is not None and len(si.on_wait) > 1:
                    changed = True
                    waits = list(si.on_wait)
                    for w in waits[:-1]:
                        _mw_ctr[0] += 1
                        nop = mybir.InstNoOp(
                            name=f"mwsplit-{_mw_ctr[0]}", ins=[], outs=[]
                        )
                        nop.engine = inst.engine
                        nop.sync_info = mybir.SyncInfo(on_wait=[w], on_update=[])
                        out.append(nop)
                    inst.sync_info = mybir.SyncInfo(
                        on_wait=[waits[-1]], on_update=list(si.on_update)
                    )
                out.append(inst)
            if changed:
                insts[:] = out


def _build_nc():
    nc = bass.Bass()
    x_d = nc.dram_tensor("x", [C, L], BF16, kind="ExternalInput")
    wv1t_d = nc.dram_tensor("wv1t", [128, 2, CB, 128], BF16, kind="ExternalInput")
    wcv_d = nc.dram_tensor("wcv", [128, 4, CB, 128], F32, kind="ExternalInput")
    wtt_d = nc.dram_tensor("wtt", [128, 2, 128], BF16, kind="ExternalInput")
    wch_d = nc.dram_tensor("wch", [128, 4, 128], F32, kind="ExternalInput")
    wct_d = nc.dram_tensor("wct", [128, CB, 128], BF16, kind="ExternalInput")
    bval_d = nc.dram_tensor("bval", [128, CB], F32, kind="ExternalInput")
    btdnn_d = nc.dram_tensor("btdnn", [128, 1], F32, kind="ExternalInput")
    bp_d = nc.dram_tensor("bp", [128, CB], F32, kind="ExternalInput")
    scal_d = nc.dram_tensor("scal", [128, 2], F32, kind="ExternalInput")
    out_d = nc.dram_tensor("out", [2 * C, 1], F32, kind="ExternalOutput")

    with tile.TileContext(nc) as tc:
        with (
            tc.tile_pool(name="consts", bufs=1) as cp,
            tc.tile_pool(name="xs", bufs=1) as xp,
            tc.tile_pool(name="hw", bufs=4) as hp,
            tc.tile_pool(name="pw", bufs=4) as pp,
            tc.tile_pool(name="pvw", bufs=4) as pvp,
            tc.tile_pool(name="pv2w", bufs=2) as pv2p,
        ):
            # ---- load weights / consts ----
            wv1t = cp.tile([128, 2, CB, 128], BF16, tag="wv1t", name="wv1t")
            nc.sync.dma_start(out=wv1t, in_=wv1t_d[:, :, :, :])
            wcv = cp.tile([128, 4, CB, 128], F32, tag="wcv", name="wcv")
            nc.sync.dma_start(out=wcv, in_=wcv_d[:, :, :, :])
            wtt = cp.tile([128, 2, 128], BF16, tag="wtt", name="wtt")
            nc.sync.dma_start(out=wtt, in_=wtt_d[:, :, :])
            wch = cp.tile([128, 4, 128], F32, tag="wch", name="wch")
            nc.sync.dma_start(out=wch, in_=wch_d[:, :, :])
            wct = cp.tile([128, CB, 128], BF16, tag="wct", name="wct")
            nc.sync.dma_start(out=wct, in_=wct_d[:, :, :])
            bval = cp.tile([128, CB], F32, tag="bval", name="bval")
            nc.sync.dma_start(out=bval, in_=bval_d[:, :])
            btdnn = cp.tile([128, 1], F32, tag="btdnn", name="btdnn")
            nc.sync.dma_start(out=btdnn, in_=btdnn_d[:, :])
            bp = cp.tile([128, CB], F32, tag="bp", name="bp")
            nc.sync.dma_start(out=bp, in_=bp_d[:, :])
            scal = cp.tile([128, 2], F32, tag="scal", name="scal")
            nc.sync.dma_start(out=scal, in_=scal_d[:, :])

            # ---- load x (chunked) + stats accumulation (all on DVE) ----
            xs = []
            sumxp = []
            sumsqp = []
            dummy_a = cp.tile([128, LD], BF16, tag="dummy_a", name="dummy_a")
            dummy_b = cp.tile([128, LD], BF16, tag="dummy_b", name="dummy_b")
            for cb in range(CB):
                xs.append([xp.tile([128, LD], BF16, tag=f"x{cb}_{j}", name=f"x{cb}_{j}")
                           for j in range(NDMA)])
                sumxp.append(cp.tile([128, NDMA], F32, tag=f"sumxp{cb}", name=f"sumxp{cb}"))
                sumsqp.append(cp.tile([128, NDMA], F32, tag=f"sumsqp{cb}", name=f"sumsqp{cb}"))
            for j in range(NDMA):
                for cb in range(CB):
                    sl = slice(j * LD, (j + 1) * LD)
                    nc.sync.dma_start(
                        out=xs[cb][j], in_=x_d[cb * 128 : (cb + 1) * 128, sl]
                    )
                    nc.vector.tensor_scalar(
                        out=dummy_a,
                        in0=xs[cb][j],
                        scalar1=1.0,
                        scalar2=0.0,
                        op0=ALU.mult,
                        op1=ALU.add,
                        accum_out=sumxp[cb][:, j : j + 1],
                    )
                    nc.scalar.activation(
                        out=dummy_b,
                        in_=xs[cb][j],
                        func=ACT.Square,
                        accum_out=sumsqp[cb][:, j : j + 1],
                    )

            # ---- finalize stats: gmean / gstd per c-block ----
            gmean, gstd = [], []
            for cb in range(CB):
                sx = cp.tile([128, 1], F32, tag=f"sx{cb}", name=f"sx{cb}")
                nc.vector.tensor_reduce(
                    out=sx, in_=sumxp[cb], axis=mybir.AxisListType.X, op=ALU.add
                )
                sq = cp.tile([128, 1], F32, tag=f"sq{cb}", name=f"sq{cb}")
                nc.vector.tensor_reduce(
                    out=sq, in_=sumsqp[cb], axis=mybir.AxisListType.X, op=ALU.add
                )
                gm = cp.tile([128, 1], F32, tag=f"gm{cb}", name=f"gm{cb}")
                nc.vector.tensor_scalar_mul(out=gm, in0=sx, scalar1=scal[:, 0:1])
                msq = cp.tile([128, 1], F32, tag=f"msq{cb}", name=f"msq{cb}")
                nc.vector.tensor_scalar_mul(out=msq, in0=sq, scalar1=scal[:, 0:1])
                gm2 = cp.tile([128, 1], F32, tag=f"gm2{cb}", name=f"gm2{cb}")
                nc.vector.tensor_mul(out=gm2, in0=gm, in1=gm)
                gv = cp.tile([128, 1], F32, tag=f"gv{cb}", name=f"gv{cb}")
                nc.vector.tensor_sub(out=gv, in0=msq, in1=gm2)
                nc.vector.tensor_scalar_max(out=gv, in0=gv, scalar1=EPS)
                # gstd = exp(0.5*ln(gv)) -- keeps ACT on one table set (ln+exp)
                lng = cp.tile([128, 1], F32, tag=f"lng{cb}", name=f"lng{cb}")
                nc.scalar.activation(out=lng, in_=gv, func=ACT.Ln)
                gs = cp.tile([128, 1], F32, tag=f"gs{cb}", name=f"gs{cb}")
                nc.scalar.activation(out=gs, in_=lng, func=ACT.Exp, scale=0.5)
                gmean.append(gm)
                gstd.append(gs)

            gg = [gmean[0], gmean[1], gstd[0], gstd[1]]  # [f'] blocks of [gmean; gstd]

            # ---- derived consts via tiny PE matvecs (fp32) ----
            with tc.tile_pool(name="pscst", bufs=2, space="PSUM") as pcst:
                ps_ch = pcst.tile([128, 1], F32, tag="pch", name="pch")
                for j in range(4):
                    nc.tensor.matmul(
                        ps_ch, lhsT=wch[:, j, :], rhs=gg[j], start=(j == 0), stop=(j == 3)
                    )
                const_h = cp.tile([128, 1], F32, tag="const_h", name="const_h")
                nc.scalar.activation(
                    out=const_h, in_=ps_ch, func=ACT.Identity, bias=btdnn[:, 0:1]
                )
                hinv = cp.tile([128, 1], BF16, tag="hinv", name="hinv")
                nc.scalar.activation(out=hinv, in_=const_h, func=ACT.Relu)

                cv, pinv = [], []
                for cb in range(CB):
                    ps_cv = pcst.tile([128, 1], F32, tag="pcv", name="pcv")
                    for j in range(4):
                        nc.tensor.matmul(
                            ps_cv,
                            lhsT=wcv[:, j, cb, :],
                            rhs=gg[j],
                            start=(j == 0),
                            stop=(j == 3),
                        )
                    cvt = cp.tile([128, 1], F32, tag=f"cv{cb}", name=f"cv{cb}")
                    nc.scalar.activation(
                        out=cvt, in_=ps_cv, func=ACT.Identity, bias=bval[:, cb : cb + 1]
                    )
                    cv.append(cvt)
                    ps_si = pcst.tile([128, 1], F32, tag="psi", name="psi")
                    nc.tensor.matmul(ps_si, lhsT=wct[:, cb, :], rhs=hinv, start=True, stop=True)
                    pv_t = cp.tile([128, 1], F32, tag=f"pinv{cb}", name=f"pinv{cb}")
                    nc.scalar.activation(
                        out=pv_t, in_=ps_si, func=ACT.Exp, bias=bp[:, cb : cb + 1]
                    )
                    pinv.append(pv_t)

            # ---- streaming phase ----
            Zp = [cp.tile([128, NCHUNK], F32, tag=f"Zp{cb}", name=f"Zp{cb}") for cb in range(CB)]
            S1p = [cp.tile([128, NCHUNK], F32, tag=f"S1p{cb}", name=f"S1p{cb}") for cb in range(CB)]
            S2p = [cp.tile([128, NCHUNK], F32, tag=f"S2p{cb}", name=f"S2p{cb}") for cb in range(CB)]

            with (
                tc.tile_pool(name="psh", bufs=2, space="PSUM") as ps_h,
                tc.tile_pool(name="psv", bufs=2, space="PSUM") as ps_v,
                tc.tile_pool(name="pss", bufs=2, space="PSUM") as ps_s,
            ):
                NSUP = NCHUNK // 2
                for s_i in range(NSUP):
                    hs = []
                    for half in range(2):
                        i = 2 * s_i + half
                        j, r = divmod(i * LC, LD)
                        xsl = [xs[cb][j][:, r : r + LC] for cb in range(CB)]
                        ph = ps_h.tile([128, LC], F32, tag="ph", name="ph")
                        nc.tensor.matmul(ph, lhsT=wtt[:, 0, :], rhs=xsl[0], start=True, stop=False)
                        nc.tensor.matmul(ph, lhsT=wtt[:, 1, :], rhs=xsl[1], start=False, stop=True)
                        h = hp.tile([128, LC], BF16, tag="h", name="h")
                        nc.scalar.activation(out=h, in_=ph, func=ACT.Relu, bias=const_h[:, 0:1])
                        hs.append(h)
                    for cb in range(CB):
                        sps = ps_s.tile([128, 2 * LC], F32, tag="scr", name="scr")
                        vpss = []
                        for half in range(2):
                            i = 2 * s_i + half
                            j, r = divmod(i * LC, LD)
                            xsl = [xs[cb2][j][:, r : r + LC] for cb2 in range(CB)]
                            vps = ps_v.tile([128, LC], F32, tag="vraw", name="vraw")
                            nc.tensor.matmul(vps, lhsT=wv1t[:, 0, cb, :], rhs=xsl[0], start=True, stop=False)
                            nc.tensor.matmul(vps, lhsT=wv1t[:, 1, cb, :], rhs=xsl[1], start=False, stop=True)
                            vpss.append(vps)
                            nc.tensor.matmul(sps[:, half * LC : (half + 1) * LC],
                                             lhsT=wct[:, cb, :], rhs=hs[half], start=True, stop=True)
                        p = pp.tile([128, 2 * LC], BF16, tag="p", name="p")
                        nc.scalar.activation(
                            out=p, in_=sps, func=ACT.Exp,
                            bias=bp[:, cb : cb + 1],
                            accum_out=Zp[cb][:, s_i : s_i + 1],
                        )
                        for half in range(2):
                            i = 2 * s_i + half
                            pv = pvp.tile([128, LC], BF16, tag="pv", name="pv")
                            nc.vector.scalar_tensor_tensor(
                                out=pv, in0=p[:, half * LC : (half + 1) * LC],
                                scalar=0.0, in1=vpss[half],
                                op0=ALU.bypass, op1=ALU.mult,
                                accum_out=S1p[cb][:, i : i + 1],
                            )
                            pv2 = pv2p.tile([128, LC], BF16, tag="pv2", name="pv2")
                            nc.vector.scalar_tensor_tensor(
                                out=pv2, in0=pv, scalar=0.0, in1=vpss[half],
                                op0=ALU.bypass, op1=ALU.mult,
                                accum_out=S2p[cb][:, i : i + 1],
                            )

            # ---- finalize ----
            for cb in range(CB):
                Z = cp.tile([128, 1], F32, tag=f"Z{cb}", name=f"Z{cb}")
                nc.vector.tensor_reduce(out=Z, in_=Zp[cb][:, : NCHUNK // 2], axis=mybir.AxisListType.X, op=ALU.add)
                corr = cp.tile([128, 1], F32, tag=f"corr{cb}", name=f"corr{cb}")
                nc.vector.tensor_scalar_mul(out=corr, in0=pinv[cb], scalar1=scal[:, 1:2])
                Zv = cp.tile([128, 1], F32, tag=f"Zv{cb}", name=f"Zv{cb}")
                nc.vector.tensor_sub(out=Zv, in0=Z, in1=corr)
                rz = cp.tile([128, 1], F32, tag=f"rz{cb}", name=f"rz{cb}")
                nc.vector.reciprocal(out=rz, in_=Zv)
                S1 = cp.tile([128, 1], F32, tag=f"S1{cb}", name=f"S1{cb}")
                nc.vector.tensor_reduce(out=S1, in_=S1p[cb], axis=mybir.AxisListType.X, op=ALU.add)
                S2 = cp.tile([128, 1], F32, tag=f"S2{cb}", name=f"S2{cb}")
                nc.vector.tensor_reduce(out=S2, in_=S2p[cb], axis=mybir.AxisListType.X, op=ALU.add)
                m1 = cp.tile([128, 1], F32, tag=f"m1{cb}", name=f"m1{cb}")
                nc.vector.tensor_scalar_mul(out=m1, in0=S1, scalar1=rz)
                amean = cp.tile([128, 1], F32, tag=f"amean{cb}", name=f"amean{cb}")
                nc.vector.tensor_add(out=amean, in0=m1, in1=cv[cb])
                t1 = cp.tile([128, 1], F32, tag=f"t1{cb}", name=f"t1{cb}")
                nc.vector.tensor_scalar_mul(out=t1, in0=S2, scalar1=rz)
                m1sq = cp.tile([128, 1], F32, tag=f"m1sq{cb}", name=f"m1sq{cb}")
                nc.vector.tensor_mul(out=m1sq, in0=m1, in1=m1)
                avar = cp.tile([128, 1], F32, tag=f"avar{cb}", name=f"avar{cb}")
                nc.vector.tensor_sub(out=avar, in0=t1, in1=m1sq)
                nc.vector.tensor_scalar_max(out=avar, in0=avar, scalar1=EPS)
                lnv = cp.tile([128, 1], F32, tag=f"lnv{cb}", name=f"lnv{cb}")
                nc.scalar.activation(out=lnv, in_=avar, func=ACT.Ln)
                astd = cp.tile([128, 1], F32, tag=f"astd{cb}", name=f"astd{cb}")
                nc.scalar.activation(out=astd, in_=lnv, func=ACT.Exp, scale=0.5)
                nc.sync.dma_start(out=out_d[cb * 128 : (cb + 1) * 128, :], in_=amean)
                nc.sync.dma_start(out=out_d[C + cb * 128 : C + (cb + 1) * 128, :], in_=astd)

    _split_multiwaits(nc)
    return nc


_NC_CACHE = None


def _get_nc():
    global _NC_CACHE
    if _NC_CACHE is None:
        _NC_CACHE = _build_nc()
    return _NC_CACHE


def _prep_inputs(x, lengths, w_val, b_val, w_tdnn, b_tdnn, bn_gamma, bn_beta,
                 w_conv, b_conv):
    x = np.asarray(x, dtype=np.float32)
    lengths = np.asarray(lengths, dtype=np.float32)
    w_val = np.asarray(w_val, dtype=np.float32)
    b_val = np.asarray(b_val, dtype=np.float32)
    w_tdnn = np.asarray(w_tdnn, dtype=np.float32)
    b_tdnn = np.asarray(b_tdnn, dtype=np.float32)
    bn_gamma = np.asarray(bn_gamma, dtype=np.float32)
    bn_beta = np.asarray(bn_beta, dtype=np.float32)
    w_conv = np.asarray(w_conv, dtype=np.float32)
    b_conv = np.asarray(b_conv, dtype=np.float32)

    mask = (np.arange(L, dtype=np.float32)[None, :] < (lengths * L)[:, None])
    total = mask.sum(axis=1).astype(np.float32)            # [B]
    xm = (x * mask[:, None, :].astype(np.float32)).astype(ml_dtypes.bfloat16)

    def pack_lhsT(w, kblocks, cblocks, dt=None):
        # w: [K, M] (contraction-major) -> [128, kblocks, cblocks, 128]
        Ktot, Mtot = w.shape
        assert Ktot == kblocks * 128 and Mtot == cblocks * 128
        r = np.ascontiguousarray(
            w.reshape(kblocks, 128, cblocks, 128).transpose(1, 0, 2, 3)
        )
        return r.astype(dt) if dt is not None else r

    W1T = w_val[:, :C].T                                   # [f, c]
    wv1t = pack_lhsT(W1T, 2, CB, ml_dtypes.bfloat16)
    Wcv = np.concatenate([w_val[:, C:2 * C].T, w_val[:, 2 * C:].T], axis=0)  # [2C, C]
    wcv = pack_lhsT(Wcv, 4, CB)
    WtT = w_tdnn[:, :C].T                                  # [f, a]
    wtt = pack_lhsT(WtT, 2, 1, ml_dtypes.bfloat16).reshape(128, 2, 128)
    Wch = np.concatenate([w_tdnn[:, C:2 * C].T, w_tdnn[:, 2 * C:].T], axis=0)
    wch = pack_lhsT(Wch, 4, 1).reshape(128, 4, 128)
    WcT = (w_conv * bn_gamma[None, :]).T                   # [a, c]
    wct = pack_lhsT(WcT, 1, CB, ml_dtypes.bfloat16).reshape(128, CB, 128)
    bprime = b_conv + w_conv @ bn_beta                     # [C]

    shared = {
        "wv1t": wv1t, "wcv": wcv, "wtt": wtt, "wch": wch, "wct": wct,
        "bval": np.ascontiguousarray(b_val.reshape(CB, 128).T),
        "btdnn": np.ascontiguousarray(b_tdnn.reshape(128, 1)),
        "bp": np.ascontiguousarray(bprime.reshape(CB, 128).T),
    }
    in_maps = []
    for b in range(B):
        m = dict(shared)
        m["x"] = np.ascontiguousarray(xm[b])
        scal = np.empty((128, 2), dtype=np.float32)
        scal[:, 0] = 1.0 / total[b]
        scal[:, 1] = L - total[b]
        m["scal"] = scal
        in_maps.append(m)
    return in_maps


def kernel(**inputs) -> np.ndarray:
    in_maps = _prep_inputs(**inputs)
    nc = _get_nc()
    res = run_bass_kernel_spmd(nc, in_maps, core_ids=list(range(B)))
    out = np.stack([res.results[b]["out"] for b in range(B)], axis=0)  # [B, 2C, 1]
    return out.astype(np.float32)


# revision 7
# speedup vs baseline: 1.9134x; 1.0768x over previous
"""Trainium2 Bass kernel for ContinuousAttentiveStatisticsPooling.

Shape config (hardcoded): B=8, C=256, L=8192, A=128, 8 NeuronCores,
pure data parallel over B (one example per core).

Math restructure (per example, x is [C, L]):
  - Host zeroes x beyond the valid length -> all L-reductions over full L
    equal masked reductions (gmean/gstd; and W @ x has exact-zero tails).
  - gmean = sum(x)/total ; gstd = sqrt(clip(sum(x^2)/total - gmean^2))
  - vraw     = W1 @ x  (no bias; amean/avar reconstructed from raw moments)
  - pre_h    = Wt1 @ x + ch,  ch = Wt2 @ gmean + Wt3 @ gstd + b_tdnn
  - h        = relu(pre_h)       (gamma folded into Wc', beta into b')
  - scores   = Wc' @ h + b'
  - p        = exp(scores)   (no max subtraction; scores are O(1))
  - The invalid tail of scores is an exactly-computable constant s_inv, so
    Z_valid = sum_L p - n_invalid * exp(s_inv + b') ; p*vraw has zero tail.
  - amean = S1/Z + cv ; avar = S2/Z - (S1/Z)^2 with
    S1 = sum p*vraw, S2 = sum p*vraw^2, cv = W2@gmean + W3@gstd + b_val.

Precision: x / matmul weights / elementwise interiors in bf16; all
reductions, biases, and the derived consts in fp32.
"""

import sys

if "/opt/trn_rl_repo" not in sys.path:
    sys.path.insert(0, "/opt/trn_rl_repo")

import numpy as np
import ml_dtypes

import concourse.bass as bass
import concourse.mybir as mybir
import concourse.tile as tile
from concourse.bass_utils import run_bass_kernel_spmd

B, C, L, A = 8, 256, 8192, 128
CB = C // 128          # 2 c-blocks
NCHUNK = 16            # streaming chunks over L
LC = L // NCHUNK       # 512
NDMA = 4               # x DMA chunks per c-block
LD = L // NDMA         # 2048
EPS = 1e-12
F32 = mybir.dt.float32
BF16 = mybir.dt.bfloat16
ALU = mybir.AluOpType
ACT = mybir.ActivationFunctionType

_mw_ctr = [0]


def _split_multiwaits(nc):
    """This walrus build supports only ONE sync-wait per instruction.
    Split multi-wait instructions into single-wait NoOps on the same engine
    (same-engine program order preserves semantics exactly)."""
    for f in nc.m.functions:
        for blk in f.blocks:
            insts = blk.instructions
            out = []
            changed = False
            for inst in insts:
                si = inst.sync_info
                if si is not None and len(si.on_wait) > 1:
                    changed = True
                    waits = list(si.on_wait)
                    for w in waits[:-1]:
                        _mw_ctr[0] += 1
                        nop = mybir.InstNoOp(
                            name=f"mwsplit-{_mw_ctr[0]}", ins=[], outs=[]
                        )
                        nop.engine = inst.engine
                        nop.sync_info = mybir.SyncInfo(on_wait=[w], on_update=[])
                        out.append(nop)
                    inst.sync_info = mybir.SyncInfo(
                        on_wait=[waits[-1]], on_update=list(si.on_update)
                    )
                out.append(inst)
            if changed:
                insts[:] = out


def _build_nc():
    nc = bass.Bass()
    x_d = nc.dram_tensor("x", [C, L], BF16, kind="ExternalInput")
    wv1t_d = nc.dram_tensor("wv1t", [128, 2, CB, 128], BF16, kind="ExternalInput")
    wcv_d = nc.dram_tensor("wcv", [128, 4, CB, 128], F32, kind="ExternalInput")
    wtt_d = nc.dram_tensor("wtt", [128, 2, 128], BF16, kind="ExternalInput")
    wch_d = nc.dram_tensor("wch", [128, 4, 128], F32, kind="ExternalInput")
    wct_d = nc.dram_tensor("wct", [128, CB, 128], BF16, kind="ExternalInput")
    bval_d = nc.dram_tensor("bval", [128, CB], F32, kind="ExternalInput")
    btdnn_d = nc.dram_tensor("btdnn", [128, 1], F32, kind="ExternalInput")
    bp_d = nc.dram_tensor("bp", [128, CB], F32, kind="ExternalInput")
    scal_d = nc.dram_tensor("scal", [128, 2], F32, kind="ExternalInput")
    out_d = nc.dram_tensor("out", [2 * C, 1], F32, kind="ExternalOutput")

    with tile.TileContext(nc) as tc:
        with (
            tc.tile_pool(name="consts", bufs=1) as cp,
            tc.tile_pool(name="xs", bufs=1) as xp,
            tc.tile_pool(name="hw", bufs=4) as hp,
            tc.tile_pool(name="pw", bufs=4) as pp,
            tc.tile_pool(name="pvw", bufs=4) as pvp,
            tc.tile_pool(name="pv2w", bufs=2) as pv2p,
        ):
            # ---- load x (chunked) + stats accumulation (all on DVE) ----
            xs = []
            sumxp = []
            sumsqp = []
            dummy_a = cp.tile([128, LD], BF16, tag="dummy_a", name="dummy_a")
            dummy_b = cp.tile([128, LD], BF16, tag="dummy_b", name="dummy_b")
            for cb in range(CB):
                xs.append([xp.tile([128, LD], BF16, tag=f"x{cb}_{j}", name=f"x{cb}_{j}")
                           for j in range(NDMA)])
                sumxp.append(cp.tile([128, NDMA], F32, tag=f"sumxp{cb}", name=f"sumxp{cb}"))
                sumsqp.append(cp.tile([128, NDMA], F32, tag=f"sumsqp{cb}", name=f"sumsqp{cb}"))
            for j in range(NDMA):
                for cb in range(CB):
                    sl = slice(j * LD, (j + 1) * LD)
                    nc.sync.dma_start(
                        out=xs[cb][j], in_=x_d[cb * 128 : (cb + 1) * 128, sl]
                    )
                    nc.vector.tensor_scalar(
                        out=dummy_a,
                        in0=xs[cb][j],
                        scalar1=1.0,
                        scalar2=0.0,
                        op0=ALU.mult,
                        op1=ALU.add,
                        accum_out=sumxp[cb][:, j : j + 1],
                    )
                    nc.scalar.activation(
                        out=dummy_b,
                        in_=xs[cb][j],
                        func=ACT.Square,
                        accum_out=sumsqp[cb][:, j : j + 1],
                    )

            # ---- load weights / consts ----
            wv1t = cp.tile([128, 2, CB, 128], BF16, tag="wv1t", name="wv1t")
            nc.sync.dma_start(out=wv1t, in_=wv1t_d[:, :, :, :])
            wcv = cp.tile([128, 4, CB, 128], F32, tag="wcv", name="wcv")
            nc.sync.dma_start(out=wcv, in_=wcv_d[:, :, :, :])
            wtt = cp.tile([128, 2, 128], BF16, tag="wtt", name="wtt")
            nc.sync.dma_start(out=wtt, in_=wtt_d[:, :, :])
            wch = cp.tile([128, 4, 128], F32, tag="wch", name="wch")
            nc.sync.dma_start(out=wch, in_=wch_d[:, :, :])
            wct = cp.tile([128, CB, 128], BF16, tag="wct", name="wct")
            nc.sync.dma_start(out=wct, in_=wct_d[:, :, :])
            bval = cp.tile([128, CB], F32, tag="bval", name="bval")
            nc.sync.dma_start(out=bval, in_=bval_d[:, :])
            btdnn = cp.tile([128, 1], F32, tag="btdnn", name="btdnn")
            nc.sync.dma_start(out=btdnn, in_=btdnn_d[:, :])
            bp = cp.tile([128, CB], F32, tag="bp", name="bp")
            nc.sync.dma_start(out=bp, in_=bp_d[:, :])
            scal = cp.tile([128, 2], F32, tag="scal", name="scal")
            nc.sync.dma_start(out=scal, in_=scal_d[:, :])

            # ---- finalize stats: gmean / gstd per c-block ----
            gmean, gstd = [], []
            for cb in range(CB):
                sx = cp.tile([128, 1], F32, tag=f"sx{cb}", name=f"sx{cb}")
                nc.vector.tensor_reduce(
                    out=sx, in_=sumxp[cb], axis=mybir.AxisListType.X, op=ALU.add
                )
                sq = cp.tile([128, 1], F32, tag=f"sq{cb}", name=f"sq{cb}")
                nc.vector.tensor_reduce(
                    out=sq, in_=sumsqp[cb], axis=mybir.AxisListType.X, op=ALU.add
                )
                gm = cp.tile([128, 1], F32, tag=f"gm{cb}", name=f"gm{cb}")
                nc.vector.tensor_scalar_mul(out=gm, in0=sx, scalar1=scal[:, 0:1])
                msq = cp.tile([128, 1], F32, tag=f"msq{cb}", name=f"msq{cb}")
                nc.vector.tensor_scalar_mul(out=msq, in0=sq, scalar1=scal[:, 0:1])
                gm2 = cp.tile([128, 1], F32, tag=f"gm2{cb}", name=f"gm2{cb}")
                nc.vector.tensor_mul(out=gm2, in0=gm, in1=gm)
                gv = cp.tile([128, 1], F32, tag=f"gv{cb}", name=f"gv{cb}")
                nc.vector.tensor_sub(out=gv, in0=msq, in1=gm2)
                nc.vector.tensor_scalar_max(out=gv, in0=gv, scalar1=EPS)
                # gstd = exp(0.5*ln(gv)) -- keeps ACT on one table set (ln+exp)
                lng = cp.tile([128, 1], F32, tag=f"lng{cb}", name=f"lng{cb}")
                nc.scalar.activation(out=lng, in_=gv, func=ACT.Ln)
                gs = cp.tile([128, 1], F32, tag=f"gs{cb}", name=f"gs{cb}")
                nc.scalar.activation(out=gs, in_=lng, func=ACT.Exp, scale=0.5)
                gmean.append(gm)
                gstd.append(gs)

            gg = [gmean[0], gmean[1], gstd[0], gstd[1]]  # [f'] blocks of [gmean; gstd]

            # ---- derived consts via tiny PE matvecs (fp32) ----
            with tc.tile_pool(name="pscst", bufs=2, space="PSUM") as pcst:
                ps_ch = pcst.tile([128, 1], F32, tag="pch", name="pch")
                for j in range(4):
                    nc.tensor.matmul(
                        ps_ch, lhsT=wch[:, j, :], rhs=gg[j], start=(j == 0), stop=(j == 3)
                    )
                const_h = cp.tile([128, 1], F32, tag="const_h", name="const_h")
                nc.scalar.activation(
                    out=const_h, in_=ps_ch, func=ACT.Identity, bias=btdnn[:, 0:1]
                )
                hinv = cp.tile([128, 1], BF16, tag="hinv", name="hinv")
                nc.scalar.activation(out=hinv, in_=const_h, func=ACT.Relu)

                cv, pinv = [], []
                for cb in range(CB):
                    ps_cv = pcst.tile([128, 1], F32, tag="pcv", name="pcv")
                    for j in range(4):
                        nc.tensor.matmul(
                            ps_cv,
                            lhsT=wcv[:, j, cb, :],
                            rhs=gg[j],
                            start=(j == 0),
                            stop=(j == 3),
                        )
                    cvt = cp.tile([128, 1], F32, tag=f"cv{cb}", name=f"cv{cb}")
                    nc.scalar.activation(
                        out=cvt, in_=ps_cv, func=ACT.Identity, bias=bval[:, cb : cb + 1]
                    )
                    cv.append(cvt)
                    ps_si = pcst.tile([128, 1], F32, tag="psi", name="psi")
                    nc.tensor.matmul(ps_si, lhsT=wct[:, cb, :], rhs=hinv, start=True, stop=True)
                    pv_t = cp.tile([128, 1], F32, tag=f"pinv{cb}", name=f"pinv{cb}")
                    nc.scalar.activation(
                        out=pv_t, in_=ps_si, func=ACT.Exp, bias=bp[:, cb : cb + 1]
                    )
                    pinv.append(pv_t)

            # ---- streaming phase ----
            Zp = [cp.tile([128, NCHUNK], F32, tag=f"Zp{cb}", name=f"Zp{cb}") for cb in range(CB)]
            S1p = [cp.tile([128, NCHUNK], F32, tag=f"S1p{cb}", name=f"S1p{cb}") for cb in range(CB)]
            S2p = [cp.tile([128, NCHUNK], F32, tag=f"S2p{cb}", name=f"S2p{cb}") for cb in range(CB)]

            with (
                tc.tile_pool(name="psh", bufs=2, space="PSUM") as ps_h,
                tc.tile_pool(name="psv", bufs=2, space="PSUM") as ps_v,
                tc.tile_pool(name="pss", bufs=1, space="PSUM") as ps_s,
            ):
                NSUP = NCHUNK // 2
                for s_i in range(NSUP):
                    hs = []
                    for half in range(2):
                        i = 2 * s_i + half
                        j, r = divmod(i * LC, LD)
                        xsl = [xs[cb][j][:, r : r + LC] for cb in range(CB)]
                        ph = ps_h.tile([128, LC], F32, tag="ph", name="ph")
                        nc.tensor.matmul(ph, lhsT=wtt[:, 0, :], rhs=xsl[0], start=True, stop=False)
                        nc.tensor.matmul(ph, lhsT=wtt[:, 1, :], rhs=xsl[1], start=False, stop=True)
                        h = hp.tile([128, LC], BF16, tag="h", name="h")
                        nc.scalar.activation(out=h, in_=ph, func=ACT.Relu, bias=const_h[:, 0:1])
                        hs.append(h)
                    for cb in range(CB):
                        sps = ps_s.tile([128, 2 * LC], F32, tag="scr", name="scr")
                        vps = ps_v.tile([128, 2 * LC], F32, tag="vraw", name="vraw")
                        for half in range(2):
                            i = 2 * s_i + half
                            j, r = divmod(i * LC, LD)
                            xsl = [xs[cb2][j][:, r : r + LC] for cb2 in range(CB)]
                            hsl = slice(half * LC, (half + 1) * LC)
                            nc.tensor.matmul(vps[:, hsl], lhsT=wv1t[:, 0, cb, :], rhs=xsl[0], start=True, stop=False)
                            nc.tensor.matmul(vps[:, hsl], lhsT=wv1t[:, 1, cb, :], rhs=xsl[1], start=False, stop=True)
                            nc.tensor.matmul(sps[:, hsl],
                                             lhsT=wct[:, cb, :], rhs=hs[half], start=True, stop=True)
                        p = pp.tile([128, 2 * LC], BF16, tag="p", name="p")
                        nc.scalar.activation(
                            out=p, in_=sps, func=ACT.Exp,
                            bias=bp[:, cb : cb + 1],
                            accum_out=Zp[cb][:, s_i : s_i + 1],
                        )
                        pv = pvp.tile([128, 2 * LC], BF16, tag="pv", name="pv")
                        nc.vector.scalar_tensor_tensor(
                            out=pv, in0=p, scalar=0.0, in1=vps,
                            op0=ALU.bypass, op1=ALU.mult,
                            accum_out=S1p[cb][:, s_i : s_i + 1],
                        )
                        pv2 = pv2p.tile([128, 2 * LC], BF16, tag="pv2", name="pv2")
                        nc.vector.scalar_tensor_tensor(
                            out=pv2, in0=pv, scalar=0.0, in1=vps,
                            op0=ALU.bypass, op1=ALU.mult,
                            accum_out=S2p[cb][:, s_i : s_i + 1],
                        )

            # ---- finalize ----
            for cb in range(CB):
                Z = cp.tile([128, 1], F32, tag=f"Z{cb}", name=f"Z{cb}")
                nc.vector.tensor_reduce(out=Z, in_=Zp[cb][:, : NCHUNK // 2], axis=mybir.AxisListType.X, op=ALU.add)
                corr = cp.tile([128, 1], F32, tag=f"corr{cb}", name=f"corr{cb}")
                nc.vector.tensor_scalar_mul(out=corr, in0=pinv[cb], scalar1=scal[:, 1:2])
                Zv = cp.tile([128, 1], F32, tag=f"Zv{cb}", name=f"Zv{cb}")
                nc.vector.tensor_sub(out=Zv, in0=Z, in1=corr)
                rz = cp.tile([128, 1], F32, tag=f"rz{cb}", name=f"rz{cb}")
                nc.vector.reciprocal(out=rz, in_=Zv)
                S1 = cp.tile([128, 1], F32, tag=f"S1{cb}", name=f"S1{cb}")
                nc.vector.tensor_reduce(out=S1, in_=S1p[cb][:, : NCHUNK // 2], axis=mybir.AxisListType.X, op=ALU.add)
                S2 = cp.tile([128, 1], F32, tag=f"S2{cb}", name=f"S2{cb}")
                nc.vector.tensor_reduce(out=S2, in_=S2p[cb][:, : NCHUNK // 2], axis=mybir.AxisListType.X, op=ALU.add)
                m1 = cp.tile([128, 1], F32, tag=f"m1{cb}", name=f"m1{cb}")
                nc.vector.tensor_scalar_mul(out=m1, in0=S1, scalar1=rz)
                amean = cp.tile([128, 1], F32, tag=f"amean{cb}", name=f"amean{cb}")
                nc.vector.tensor_add(out=amean, in0=m1, in1=cv[cb])
                t1 = cp.tile([128, 1], F32, tag=f"t1{cb}", name=f"t1{cb}")
                nc.vector.tensor_scalar_mul(out=t1, in0=S2, scalar1=rz)
                m1sq = cp.tile([128, 1], F32, tag=f"m1sq{cb}", name=f"m1sq{cb}")
                nc.vector.tensor_mul(out=m1sq, in0=m1, in1=m1)
                avar = cp.tile([128, 1], F32, tag=f"avar{cb}", name=f"avar{cb}")
                nc.vector.tensor_sub(out=avar, in0=t1, in1=m1sq)
                nc.vector.tensor_scalar_max(out=avar, in0=avar, scalar1=EPS)
                lnv = cp.tile([128, 1], F32, tag=f"lnv{cb}", name=f"lnv{cb}")
                nc.scalar.activation(out=lnv, in_=avar, func=ACT.Ln)
                astd = cp.tile([128, 1], F32, tag=f"astd{cb}", name=f"astd{cb}")
                nc.scalar.activation(out=astd, in_=lnv, func=ACT.Exp, scale=0.5)
                nc.sync.dma_start(out=out_d[cb * 128 : (cb + 1) * 128, :], in_=amean)
                nc.sync.dma_start(out=out_d[C + cb * 128 : C + (cb + 1) * 128, :], in_=astd)

    _split_multiwaits(nc)
    return nc


_NC_CACHE = None


def _get_nc():
    global _NC_CACHE
    if _NC_CACHE is None:
        _NC_CACHE = _build_nc()
    return _NC_CACHE


def _prep_inputs(x, lengths, w_val, b_val, w_tdnn, b_tdnn, bn_gamma, bn_beta,
                 w_conv, b_conv):
    x = np.asarray(x, dtype=np.float32)
    lengths = np.asarray(lengths, dtype=np.float32)
    w_val = np.asarray(w_val, dtype=np.float32)
    b_val = np.asarray(b_val, dtype=np.float32)
    w_tdnn = np.asarray(w_tdnn, dtype=np.float32)
    b_tdnn = np.asarray(b_tdnn, dtype=np.float32)
    bn_gamma = np.asarray(bn_gamma, dtype=np.float32)
    bn_beta = np.asarray(bn_beta, dtype=np.float32)
    w_conv = np.asarray(w_conv, dtype=np.float32)
    b_conv = np.asarray(b_conv, dtype=np.float32)

    mask = (np.arange(L, dtype=np.float32)[None, :] < (lengths * L)[:, None])
    total = mask.sum(axis=1).astype(np.float32)            # [B]
    xm = (x * mask[:, None, :].astype(np.float32)).astype(ml_dtypes.bfloat16)

    def pack_lhsT(w, kblocks, cblocks, dt=None):
        # w: [K, M] (contraction-major) -> [128, kblocks, cblocks, 128]
        Ktot, Mtot = w.shape
        assert Ktot == kblocks * 128 and Mtot == cblocks * 128
        r = np.ascontiguousarray(
            w.reshape(kblocks, 128, cblocks, 128).transpose(1, 0, 2, 3)
        )
        return r.astype(dt) if dt is not None else r

    W1T = w_val[:, :C].T                                   # [f, c]
    wv1t = pack_lhsT(W1T, 2, CB, ml_dtypes.bfloat16)
    Wcv = np.concatenate([w_val[:, C:2 * C].T, w_val[:, 2 * C:].T], axis=0)  # [2C, C]
    wcv = pack_lhsT(Wcv, 4, CB)
    WtT = w_tdnn[:, :C].T                                  # [f, a]
    wtt = pack_lhsT(WtT, 2, 1, ml_dtypes.bfloat16).reshape(128, 2, 128)
    Wch = np.concatenate([w_tdnn[:, C:2 * C].T, w_tdnn[:, 2 * C:].T], axis=0)
    wch = pack_lhsT(Wch, 4, 1).reshape(128, 4, 128)
    WcT = (w_conv * bn_gamma[None, :]).T                   # [a, c]
    wct = pack_lhsT(WcT, 1, CB, ml_dtypes.bfloat16).reshape(128, CB, 128)
    bprime = b_conv + w_conv @ bn_beta                     # [C]

    shared = {
        "wv1t": wv1t, "wcv": wcv, "wtt": wtt, "wch": wch, "wct": wct,
        "bval": np.ascontiguousarray(b_val.reshape(CB, 128).T),
        "btdnn": np.ascontiguousarray(b_tdnn.reshape(128, 1)),
        "bp": np.ascontiguousarray(bprime.reshape(CB, 128).T),
    }
    in_maps = []
    for b in range(B):
        m = dict(shared)
        m["x"] = np.ascontiguousarray(xm[b])
        scal = np.empty((128, 2), dtype=np.float32)
        scal[:, 0] = 1.0 / total[b]
        scal[:, 1] = L - total[b]
        m["scal"] = scal
        in_maps.append(m)
    return in_maps


def kernel(**inputs) -> np.ndarray:
    in_maps = _prep_inputs(**inputs)
    nc = _get_nc()
    res = run_bass_kernel_spmd(nc, in_maps, core_ids=list(range(B)))
    out = np.stack([res.results[b]["out"] for b in range(B)], axis=0)  # [B, 2C, 1]
    return out.astype(np.float32)
